# revision 45
# baseline (speedup 1.0000x reference)
"""Multi-head attention TRN2 kernel (v2, bf16 dataflow).

Problem: B=2, T=S=2048, D=1024, H=16, DK=64 (fp32 in/out).

Sharding (8 cores): core i handles batch b = i // 4 and the 4 heads
[4*(i%4), 4*(i%4)+4).  Each core computes q/k/v projections for its head
slice, attention over them, and a *partial* output projection (its heads'
rows of Wo).  The host sums the 4 partials per batch and adds bo.

v2 design (vs v1):
  - everything bf16 except psum accumulation (f32): halves DMA, enables
    1024-wide moving operands.
  - attnV in natural layout out[t, dk+1] (M=128 per matmul, N=65), with a
    ones-column in v giving sum(exp) for free in column 64.
  - probs = exp(scores) emitted bf16 straight to SBUF by the Act engine.
  - attn normalization on DVE (reciprocal of col 64, broadcast multiply).
  - attn[t, hk] -> attnT[hk, t] via DMA xbar transpose (dma_start_transpose).
  - PE instruction stream hand-woven: each scores tile is followed by the
    attnV matmuls of the tile 4 positions back plus token-bucket-paced
    "aux" units (projections, v-proj, out-proj) so the in-order PE queue
    never stalls behind the Act engine's exp drain.

Per-core layout (host pre-transposes / pre-slices / pre-scales / casts):
  xqT, xkT, xvT : (1024, 2048) bf16  -- x[b].T
  wq, wk, wv    : (1024, 256) bf16   -- W[:, h0:h0+4, :] (wq,bq pre-scaled)
  wo            : (256, 1024) bf16   -- Wo[h0:h0+4]
  out           : (2048, 1024) bf16  -- partial; host sums in f32, adds bo
"""

import numpy as np

B, T, S, D, H, DK = 2, 2048, 2048, 1024, 16, 64
HPC = 4            # heads per core
HD = HPC * DK      # 256 projected cols per core
N_CORES = 8
DC = D // 128      # 8 contraction chunks
SC16 = S // 128    # 16 s-chunks of 128
NJ = 2             # t-chunks of 1024 ("j blocks")
TT = 8             # t-subtiles of 128 per j block
LAG = 4            # attnV trails scores by this many s-chunks
PT_BUFS = 40       # probs ring (2 blocks + margin)
AUX_TILE = 1150    # aux matmul rows budget per scores tile
AUX_CAP = 3000


def build_core(has_bias=False, debug=False):
    import concourse.mybir as mybir
    from concourse import bacc
    from concourse.tile import TileContext
    from collections import deque

    dt = mybir.dt
    f32 = dt.float32
    bf16 = dt.bfloat16
    AF = mybir.ActivationFunctionType

    nc = bacc.Bacc("TRN2", target_bir_lowering=False, debug=False,
                   num_devices=N_CORES)

    xqT = nc.dram_tensor("xqT", [D, T], bf16, kind="ExternalInput")
    xkT = nc.dram_tensor("xkT", [D, T], bf16, kind="ExternalInput")
    xvT = nc.dram_tensor("xvT", [D, T], bf16, kind="ExternalInput")
    wq = nc.dram_tensor("wq", [D, HD], bf16, kind="ExternalInput")
    wk = nc.dram_tensor("wk", [D, HD], bf16, kind="ExternalInput")
    wv = nc.dram_tensor("wv", [D, HD], bf16, kind="ExternalInput")
    wo = nc.dram_tensor("wo", [HD, D], bf16, kind="ExternalInput")
    if has_bias:
        bqs = nc.dram_tensor("bqs", [HD], bf16, kind="ExternalInput")
        bks = nc.dram_tensor("bks", [HD], bf16, kind="ExternalInput")
        bvs = nc.dram_tensor("bvs", [HD], bf16, kind="ExternalInput")
    out = nc.dram_tensor("out", [T, D], bf16, kind="ExternalOutput")
    if debug:
        dbg = {
            nm: nc.dram_tensor(nm, shp, bf16, kind="ExternalOutput")
            for nm, shp in [
                ("dkT", [128, 2, T]), ("dqT", [128, 2, T]),
                ("dv1", [128, SC16, HPC, DK + 1]), ("daT", [128, 2, T]),
            ]}

    xq_r = xqT.ap().rearrange("(c p) t -> c p t", p=128)
    xk_r = xkT.ap().rearrange("(c p) t -> c p t", p=128)
    xv_r = xvT.ap().rearrange("(c p) t -> c p t", p=128)
    wq_r = wq.ap().rearrange("(c p) n -> p c n", p=128)
    wk_r = wk.ap().rearrange("(c p) n -> p c n", p=128)
    wv_r = wv.ap().rearrange("(c p) n -> p c n", p=128)
    wo_r = wo.ap().rearrange("(c p) n -> p c n", p=128)

    with TileContext(nc) as tc:
        tc.cur_priority = 2_000_000   # default band: aux/proj/outproj/DMA
        with (
            tc.tile_pool(name="persist", bufs=1) as pp,
            tc.tile_pool(name="xin", bufs=8) as xpool,
            tc.tile_pool(name="probs", bufs=PT_BUFS) as ppool,
            tc.tile_pool(name="anat", bufs=2) as apool,
            tc.tile_pool(name="small", bufs=4) as spool,
            tc.tile_pool(name="ostage", bufs=5) as opool,
            tc.tile_pool(name="ps", bufs=1, space="PSUM") as ps,
        ):
            # ---- persistent SBUF tensors ----
            wq_sb = pp.tile([128, DC, HD], bf16)
            wk_sb = pp.tile([128, DC, HD], bf16)
            wv_sb = pp.tile([128, DC, HD], bf16)
            wo_sb = pp.tile([128, 2, D], bf16)
            qT_sb = pp.tile([128, 2, T], bf16)   # [hd-in-pair, pair, t]
            kT_sb = pp.tile([128, 2, T], bf16)
            v1_sb = pp.tile([128, SC16, HPC, DK + 1], bf16)  # [s, sc, h, dk|1]
            aT_sb = pp.tile([128, 2, T], bf16)   # [hk-in-pair, pair, t]

            # first weights (DMA order = SP emission order)
            nc.sync.dma_start(out=wk_sb[:], in_=wk_r)
            nc.sync.dma_start(out=wq_sb[:], in_=wq_r)

            # ones column of v1 (sum-exp trick)
            nc.vector.memset(v1_sb[:, :, :, DK:DK + 1], 1.0)

            if has_bias:
                bq_sb = pp.tile([1, HD], bf16)
                bk_sb = pp.tile([1, HD], bf16)
                bv_sb = pp.tile([1, HD], bf16)
                ones_sb = pp.tile([1, 512], bf16)
                nc.sync.dma_start(out=bq_sb[0:1, :], in_=bqs.ap()[None, :])
                nc.sync.dma_start(out=bk_sb[0:1, :], in_=bks.ap()[None, :])
                nc.sync.dma_start(out=bv_sb[0:1, :], in_=bvs.ap()[None, :])
                nc.vector.memset(ones_sb[:], 1.0)
            b_sb = {"q": bq_sb, "k": bk_sb} if has_bias else {"q": None,
                                                             "k": None}

            # ---- x input tiles: [128, 1024] halves ----
            # xk: ring of 8, half1 recycles half0 (DMA emitted after the
            # half0 readers).  xq: 16 tiles, no recycling.  xv: ring of 8,
            # half1 recycles half0 (DMA emitted via aux unit after the v
            # units that read half0).
            xk_t = {0: [xpool.tile([128, 1024], bf16, tag="xk", bufs=8,
                                   name=f"xk0_{c}") for c in range(DC)]}
            xq_t = {0: [xpool.tile([128, 1024], bf16, tag="xq", bufs=8,
                                   name=f"xq0_{c}") for c in range(DC)]}
            for c in range(DC):
                nc.sync.dma_start(out=xk_t[0][c][:], in_=xk_r[c][:, 0:1024])
            for c in range(DC):
                nc.sync.dma_start(out=xq_t[0][c][:], in_=xq_r[c][:, 0:1024])

            # ---------------- emission units ----------------

            def proj_qk_unit(kind, p, tch, late=False):
                # one [128hd, 512t] psum tile of the q/k projection
                w_sb, dst_sb = ((wq_sb, qT_sb) if kind == "q"
                                else (wk_sb, kT_sb))
                def _emit():
                    pt = ps.tile([128, 512], f32, tag="mix", bufs=2,
                                 name="pqk")
                    xh = (xq_t if kind == "q" else xk_t)[tch // 2]
                    lsl = slice((tch % 2) * 512, (tch % 2) * 512 + 512)
                    tsl = slice(tch * 512, (tch + 1) * 512)
                    for c in range(DC):
                        nc.tensor.matmul(
                            pt[:],
                            w_sb[:, c, p * 128:(p + 1) * 128],
                            xh[c][:, lsl],
                            start=(c == 0),
                            stop=(c == DC - 1) and not has_bias,
                        )
                    if has_bias:
                        nc.tensor.matmul(
                            pt[:],
                            b_sb[kind][0:1, p * 128:(p + 1) * 128],
                            ones_sb[0:1, :],
                            start=False, stop=True,
                        )
                    nc.vector.tensor_copy(dst_sb[:, p, tsl], pt[:])

                def emit():
                    band = "c" if late else "p"
                    with tc.high_priority(offset=tc.cur_priority - prio[band]):
                        prio[band] += 32
                        _emit()
                return emit

            def proj_v_unit(sc):
                # one [128s, 256hd] psum tile of the v projection
                def _emit():
                    pt = ps.tile([128, HD], f32, tag="mix", bufs=2, name="pv")
                    xh = xv_t[sc // 8]
                    ssl = slice((sc % 8) * 128, (sc % 8) * 128 + 128)
                    for c in range(DC):
                        nc.tensor.matmul(
                            pt[:],
                            xh[c][:, ssl],
                            wv_sb[:, c, :],
                            start=(c == 0),
                            stop=(c == DC - 1) and not has_bias,
                        )
                    if has_bias:
                        nc.tensor.matmul(
                            pt[:], ones_sb[0:1, 0:128], bv_sb[0:1, :],
                            start=False, stop=True,
                        )
                    nc.vector.tensor_copy(
                        v1_sb[:, sc, :, 0:DK],
                        pt[:].rearrange("p (h k) -> p h k", h=HPC))

                def emit():
                    with tc.high_priority(offset=tc.cur_priority - prio["p"]):
                        prio["p"] += 32
                        _emit()
                return emit

            pts = {}     # probs tiles keyed (h, j, sc)
            att4 = {}    # psum accumulators keyed (h, j, q)
            anat = {}    # normalized attn tiles keyed (pair, j)

            prio = {"s": 0, "p": 500_000, "c": 1_000_000, "l": 1_500_000}

            def scores_unit(h, j, sc):
                # [128s, 1024t] scores psum tile + its exp; band-0 priority
                # so the scheduler always prefers feeding the Act engine
                p, o = h // 2, (h % 2) * 64
                with tc.high_priority(offset=tc.cur_priority - prio["s"]):
                    prio["s"] += 32
                    st = ps.tile([128, 1024], f32, tag="sc", bufs=2,
                                 name="st")
                    for th in range(2):
                        tsl = slice(j * 1024 + th * 512,
                                    j * 1024 + (th + 1) * 512)
                        nc.tensor.matmul(
                            st[:, th * 512:(th + 1) * 512],
                            kT_sb[o:o + 64, p, sc * 128:(sc + 1) * 128],
                            qT_sb[o:o + 64, p, tsl],
                            start=True, stop=True,
                        )
                    pt = ppool.tile([128, 1024], bf16, tag="pt", name="pt")
                    nc.scalar.activation(pt[:], st[:], AF.Exp)
                    pts[(h, j, sc)] = pt

            def attnv_chain(h, j, q, tt):
                # one full accumulation chain (16 matmuls) for t-subtile
                # q*4+tt.  Chains in the same psum tile must be sequential:
                # start_tensor_calc marks the whole 2KB zero-region pending,
                # so interleaved chains corrupt each other.
                def _emit():
                    if tt == 0:
                        att4[(h, j, q)] = ps.tile(
                            [128, 4, DK + 1], f32, tag="a4", bufs=2,
                            name="att4")
                    a4 = att4[(h, j, q)]
                    for sc in range(SC16):
                        nc.tensor.matmul(
                            a4[:, tt],
                            pts[(h, j, sc)][:, (q * 4 + tt) * 128:
                                            (q * 4 + tt + 1) * 128],
                            v1_sb[:, sc, h, :],
                            start=(sc == 0),
                            stop=(sc == SC16 - 1),
                        )
                    if (q, tt) == (1, 3):
                        for sc in range(SC16):
                            del pts[(h, j, sc)]

                def emit():
                    with tc.high_priority(offset=tc.cur_priority - prio["c"]):
                        prio["c"] += 32
                        _emit()
                return emit

            def norm_unit(h, j, q):
                # normalize one att4 half of (h, j) into anat[(pair, j)]
                pair, hi = h // 2, h % 2
                def _emit():
                    if (pair, j) not in anat:
                        anat[(pair, j)] = apool.tile(
                            [128, TT, 2, DK], bf16, tag="an", name="an")
                    an = anat[(pair, j)]
                    a4 = att4.pop((h, j, q))
                    rec = spool.tile([128, 4], f32, tag="rec", name="rec")
                    nc.vector.reciprocal(rec[:], a4[:, :, DK])
                    nc.vector.tensor_mul(
                        an[:, q * 4:(q + 1) * 4, hi, :],
                        a4[:, :, 0:DK],
                        rec[:, :, None].broadcast_to([128, 4, DK]),
                    )

                def emit():
                    with tc.high_priority(offset=tc.cur_priority - prio["c"]):
                        prio["c"] += 32
                        _emit()
                return emit

            def transpose_unit(pair, j):
                # 8 dma xbar transposes [128t,128hk] -> aT[hk, t]
                def emit():
                    an = anat.pop((pair, j))
                    for tt in range(TT):
                        nc.sync.dma_start(
                            out=aT_sb[:, pair,
                                      j * 1024 + tt * 128:
                                      j * 1024 + (tt + 1) * 128],
                            in_=an[:, tt, :, :],
                            transpose=True,
                        )
                return emit

            def outproj_unit(j, ti):
                # one t-tile of the output projection: [128t, 1024d]
                tg = j * TT + ti
                tag = "sc" if (j == 1 and ti % 2 == 1) else "mix"
                def emit():
                    po = ps.tile([128, 512], f32, tag=tag, bufs=2,
                                 name="po")
                    po2 = ps.tile([128, 512], f32, tag=tag, bufs=2,
                                  name="po2")
                    for dh, pot in ((0, po), (1, po2)):
                        for hp in range(2):
                            nc.tensor.matmul(
                                pot[:],
                                aT_sb[:, hp, tg * 128:(tg + 1) * 128],
                                wo_sb[:, hp, dh * 512:(dh + 1) * 512],
                                start=(hp == 0), stop=(hp == 1),
                            )
                    ob = opool.tile([128, D], bf16, tag="ob", name="ob")
                    nc.vector.tensor_copy(ob[:, 0:512], po[:])
                    if j == 1:
                        # Act engine is idle in the tail; split the drain
                        nc.scalar.copy(ob[:, 512:1024], po2[:])
                    else:
                        nc.vector.tensor_copy(ob[:, 512:1024], po2[:])
                    nc.sync.dma_start(
                        out=out.ap()[tg * 128:(tg + 1) * 128, :], in_=ob[:])
                return emit

            # ---------------- aux queue with pacing ----------------
            aux = deque()          # entries: (cost, name, emit_fn)
            emitted = set()
            budget = [0]

            def pump(n_rows):
                budget[0] = min(budget[0] + n_rows, AUX_CAP)
                while aux and aux[0][0] <= budget[0]:
                    cost, name, fn = aux.popleft()
                    budget[0] -= cost
                    emitted.add(name)
                    fn()

            def flush_until(name):
                if name in emitted:
                    return
                while aux:
                    cost, nm, fn = aux.popleft()
                    emitted.add(nm)
                    fn()
                    if nm == name:
                        return
                raise AssertionError(f"aux marker {name} not found")

            # ---------------- the stream ----------------
            # P1/P2: k proj t0/t1 (xk half0; pair1 last -- band-P priority
            # order lets pair0 + q p0 feed block 0 first), q p0 j0
            for tch in range(2):
                proj_qk_unit("k", 0, tch)()
            for tch in range(2):
                proj_qk_unit("q", 0, tch)()
            for tch in range(2):
                proj_qk_unit("k", 1, tch)()

            # xk half1 (recycles half0 slots -- emitted after readers above),
            # then xq j1, wv, xv (both halves, own slots), wo: everything
            # up-front in consumer order, no deferred DMAs.
            xk_t[1] = [xpool.tile([128, 1024], bf16, tag="xk", bufs=8,
                                  name=f"xk1_{c}") for c in range(DC)]
            for c in range(DC):
                nc.sync.dma_start(out=xk_t[1][c][:], in_=xk_r[c][:, 1024:2048])
            nc.sync.dma_start(out=wv_sb[:], in_=wv_r)
            xv_t = {h: [xpool.tile([128, 1024], bf16, tag="xv", bufs=16,
                                   name=f"xv{h}_{c}") for c in range(DC)]
                    for h in range(2)}
            for h in range(2):
                for c in range(DC):
                    nc.sync.dma_start(out=xv_t[h][c][:],
                                      in_=xv_r[c][:, h * 1024:(h + 1) * 1024])
            nc.sync.dma_start(out=wo_sb[:], in_=wo_r)

            # aux for block 0 and the early projections
            aux.append((4096, "qp1t0", proj_qk_unit("q", 1, 0)))
            aux.append((4096, "qp1t1", proj_qk_unit("q", 1, 1)))
            for sc in range(SC16):
                aux.append((2048, f"v{sc}", proj_v_unit(sc)))
            aux.append((4096, "qp0t2", proj_qk_unit("q", 0, 2, late=True)))
            aux.append((4096, "qp0t3", proj_qk_unit("q", 0, 3, late=True)))
            aux.append((4096, "qp1t2", proj_qk_unit("q", 1, 2, late=True)))
            aux.append((4096, "qp1t3", proj_qk_unit("q", 1, 3, late=True)))

            blocks = [(0, 0), (1, 0), (2, 0), (3, 0),
                      (0, 1), (1, 1), (2, 1), (3, 1)]
            chains = [(q, tt) for q in range(2) for tt in range(4)]

            def post_block(ph, pj):
                # prev block's chains are done: normalize the q1 half (q0
                # was normalized in-loop), then transpose / out-project when
                # a (pair, j) group completes
                aux.appendleft((0, f"norm_{ph}_{pj}_1", norm_unit(ph, pj, 1)))
                if ph % 2 == 1:
                    pair = ph // 2
                    aux.append((0, f"T_{pair}_{pj}",
                                transpose_unit(pair, pj)))
                    if pair == 1:
                        for ti in range(TT):
                            aux.append((2048, f"O_{pj}_{ti}",
                                        outproj_unit(pj, ti)))

            for bi, (h, j) in enumerate(blocks):
                # markers that must be emitted before this block's scores
                if bi == 2:
                    flush_until("qp1t1")   # also drains kp1t0..3
                if bi == 4:
                    flush_until("qp0t3")
                if bi == 6:
                    flush_until("qp1t3")
                for sc in range(SC16):
                    scores_unit(h, j, sc)
                    if bi == 0 and sc == 7:
                        # k proj t2/t3 (xk half1) before sc8 needs them
                        for tch in range(2, 4):
                            proj_qk_unit("k", 0, tch)()
                        for tch in range(2, 4):
                            proj_qk_unit("k", 1, tch)()
                        continue
                    lo = TT if bi == 1 else 4
                    if bi > 0 and lo <= sc < lo + TT:
                        # weave prev block's attnV chains into the scores
                        # stream (early in the block so the probs ring and
                        # att4 tiles free well before the next seam; block 1
                        # waits for sc8 because v1 lands late)
                        if bi == 1 and sc == TT:
                            flush_until("v15")  # chains read all of v1
                        ph, pj = blocks[bi - 1]
                        q, tt = chains[sc - lo]
                        if (q, tt) == (1, 0):
                            # free the q0 psum tile before q1's conclude
                            norm_unit(ph, pj, 0)()
                        attnv_chain(ph, pj, q, tt)()
                    pump(1500 if bi <= 1 else 1024)
                if bi == 0:
                    # xq j1 recycles the xk ring (readers: k proj, emitted
                    # above); not needed until block 4
                    xq_t[1] = [xpool.tile([128, 1024], bf16, tag="xk",
                                          bufs=8, name=f"xq1_{c}")
                               for c in range(DC)]
                    for c in range(DC):
                        nc.sync.dma_start(out=xq_t[1][c][:],
                                          in_=xq_r[c][:, 1024:2048])
                if bi > 0:
                    post_block(*blocks[bi - 1])

            # tail: last block's chains + norm + transpose + outproj
            for q, tt in chains:
                if (q, tt) == (1, 0):
                    norm_unit(3, 1, 0)()
                attnv_chain(3, 1, q, tt)()
            post_block(3, 1)
            while aux:
                _, nm, fn = aux.popleft()
                emitted.add(nm)
                fn()

            if debug:
                nc.sync.dma_start(out=dbg["dkT"].ap(), in_=kT_sb[:])
                nc.sync.dma_start(out=dbg["dqT"].ap(), in_=qT_sb[:])
                nc.sync.dma_start(out=dbg["dv1"].ap(), in_=v1_sb[:])
                nc.sync.dma_start(out=dbg["daT"].ap(), in_=aT_sb[:])

    nc.compile()
    return nc


_NC_CACHE = {}


def get_nc(has_bias=False):
    key = ("nc", has_bias)
    if key not in _NC_CACHE:
        _NC_CACHE[key] = build_core(has_bias)
    return _NC_CACHE[key]


def make_in_maps(query, value, key, Wq, bq, Wk, bk, Wv, bv, Wo, bo,
                 has_bias):
    import ml_dtypes
    bf16 = ml_dtypes.bfloat16
    scale = np.float32(1.0 / np.sqrt(DK))
    xT = {}
    for b in range(B):
        xT[b] = {
            "q": np.ascontiguousarray(
                np.asarray(query[b], np.float32).T.astype(bf16)),
            "k": np.ascontiguousarray(
                np.asarray(key[b], np.float32).T.astype(bf16)),
            "v": np.ascontiguousarray(
                np.asarray(value[b], np.float32).T.astype(bf16)),
        }
    Wq_f = (np.asarray(Wq, np.float32) * scale).reshape(D, H * DK).astype(bf16)
    Wk_f = np.asarray(Wk, np.float32).reshape(D, H * DK).astype(bf16)
    Wv_f = np.asarray(Wv, np.float32).reshape(D, H * DK).astype(bf16)
    Wo_f = np.asarray(Wo, np.float32).reshape(H * DK, D).astype(bf16)
    bq_f = (np.asarray(bq, np.float32) * scale).reshape(H * DK).astype(bf16)
    bk_f = np.asarray(bk, np.float32).reshape(H * DK).astype(bf16)
    bv_f = np.asarray(bv, np.float32).reshape(H * DK).astype(bf16)
    in_maps = []
    for i in range(N_CORES):
        b = i // 4
        sl = slice((i % 4) * HD, (i % 4 + 1) * HD)
        m = {
            "xqT": xT[b]["q"],
            "xkT": xT[b]["k"],
            "xvT": xT[b]["v"],
            "wq": np.ascontiguousarray(Wq_f[:, sl]),
            "wk": np.ascontiguousarray(Wk_f[:, sl]),
            "wv": np.ascontiguousarray(Wv_f[:, sl]),
            "wo": np.ascontiguousarray(Wo_f[sl, :]),
        }
        if has_bias:
            m["bqs"] = np.ascontiguousarray(bq_f[sl])
            m["bks"] = np.ascontiguousarray(bk_f[sl])
            m["bvs"] = np.ascontiguousarray(bv_f[sl])
        in_maps.append(m)
    return in_maps


def gather(results, bo):
    out = np.zeros((B, T, D), np.float32)
    for i in range(N_CORES):
        out[i // 4] += np.asarray(results[i]["out"], np.float32)
    out += np.asarray(bo, np.float32)[None, None, :]
    return out


def kernel(query, value, key, Wq, bq, Wk, bk, Wv, bv, Wo, bo):
    from concourse.bass_utils import run_bass_kernel_spmd

    has_bias = bool(
        np.any(np.asarray(bq)) or np.any(np.asarray(bk))
        or np.any(np.asarray(bv)))
    nc = get_nc(has_bias)
    in_maps = make_in_maps(query, value, key, Wq, bq, Wk, bk, Wv, bv, Wo, bo,
                           has_bias)
    res = run_bass_kernel_spmd(nc, in_maps, list(range(N_CORES)))
    return gather(res.results, bo)


# revision 46
# speedup vs baseline: 1.0087x; 1.0087x over previous
"""Multi-head attention TRN2 kernel (v2, bf16 dataflow).

Problem: B=2, T=S=2048, D=1024, H=16, DK=64 (fp32 in/out).

Sharding (8 cores): core i handles batch b = i // 4 and the 4 heads
[4*(i%4), 4*(i%4)+4).  Each core computes q/k/v projections for its head
slice, attention over them, and a *partial* output projection (its heads'
rows of Wo).  The host sums the 4 partials per batch and adds bo.

v2 design (vs v1):
  - everything bf16 except psum accumulation (f32): halves DMA, enables
    1024-wide moving operands.
  - attnV in natural layout out[t, dk+1] (M=128 per matmul, N=65), with a
    ones-column in v giving sum(exp) for free in column 64.
  - probs = exp(scores) emitted bf16 straight to SBUF by the Act engine.
  - attn normalization on DVE (reciprocal of col 64, broadcast multiply).
  - attn[t, hk] -> attnT[hk, t] via DMA xbar transpose (dma_start_transpose).
  - PE instruction stream hand-woven: each scores tile is followed by the
    attnV matmuls of the tile 4 positions back plus token-bucket-paced
    "aux" units (projections, v-proj, out-proj) so the in-order PE queue
    never stalls behind the Act engine's exp drain.

Per-core layout (host pre-transposes / pre-slices / pre-scales / casts):
  xqT, xkT, xvT : (1024, 2048) bf16  -- x[b].T
  wq, wk, wv    : (1024, 256) bf16   -- W[:, h0:h0+4, :] (wq,bq pre-scaled)
  wo            : (256, 1024) bf16   -- Wo[h0:h0+4]
  out           : (2048, 1024) bf16  -- partial; host sums in f32, adds bo
"""

import numpy as np

B, T, S, D, H, DK = 2, 2048, 2048, 1024, 16, 64
HPC = 4            # heads per core
HD = HPC * DK      # 256 projected cols per core
N_CORES = 8
DC = D // 128      # 8 contraction chunks
SC16 = S // 128    # 16 s-chunks of 128
NJ = 2             # t-chunks of 1024 ("j blocks")
TT = 8             # t-subtiles of 128 per j block
LAG = 4            # attnV trails scores by this many s-chunks
PT_BUFS = 40       # probs ring (2 blocks + margin)
AUX_TILE = 1150    # aux matmul rows budget per scores tile
AUX_CAP = 3000


def build_core(has_bias=False, debug=False):
    import concourse.mybir as mybir
    from concourse import bacc
    from concourse.tile import TileContext
    from collections import deque

    dt = mybir.dt
    f32 = dt.float32
    bf16 = dt.bfloat16
    AF = mybir.ActivationFunctionType

    nc = bacc.Bacc("TRN2", target_bir_lowering=False, debug=False,
                   num_devices=N_CORES)

    xqT = nc.dram_tensor("xqT", [D, T], bf16, kind="ExternalInput")
    xkT = nc.dram_tensor("xkT", [D, T], bf16, kind="ExternalInput")
    xvT = nc.dram_tensor("xvT", [D, T], bf16, kind="ExternalInput")
    wq = nc.dram_tensor("wq", [D, HD], bf16, kind="ExternalInput")
    wk = nc.dram_tensor("wk", [D, HD], bf16, kind="ExternalInput")
    wv = nc.dram_tensor("wv", [D, HD], bf16, kind="ExternalInput")
    wo = nc.dram_tensor("wo", [HD, D], bf16, kind="ExternalInput")
    if has_bias:
        bqs = nc.dram_tensor("bqs", [HD], bf16, kind="ExternalInput")
        bks = nc.dram_tensor("bks", [HD], bf16, kind="ExternalInput")
        bvs = nc.dram_tensor("bvs", [HD], bf16, kind="ExternalInput")
    out = nc.dram_tensor("out", [T, D], bf16, kind="ExternalOutput")
    if debug:
        dbg = {
            nm: nc.dram_tensor(nm, shp, bf16, kind="ExternalOutput")
            for nm, shp in [
                ("dkT", [128, 2, T]), ("dqT", [128, 2, T]),
                ("dv1", [128, SC16, HPC, DK + 1]), ("daT", [128, 2, T]),
            ]}

    xq_r = xqT.ap().rearrange("(c p) t -> c p t", p=128)
    xk_r = xkT.ap().rearrange("(c p) t -> c p t", p=128)
    xv_r = xvT.ap().rearrange("(c p) t -> c p t", p=128)
    wq_r = wq.ap().rearrange("(c p) n -> p c n", p=128)
    wk_r = wk.ap().rearrange("(c p) n -> p c n", p=128)
    wv_r = wv.ap().rearrange("(c p) n -> p c n", p=128)
    wo_r = wo.ap().rearrange("(c p) n -> p c n", p=128)

    with TileContext(nc) as tc:
        tc.cur_priority = 2_000_000   # default band: aux/proj/outproj/DMA
        with (
            tc.tile_pool(name="persist", bufs=1) as pp,
            tc.tile_pool(name="xin", bufs=8) as xpool,
            tc.tile_pool(name="probs", bufs=PT_BUFS) as ppool,
            tc.tile_pool(name="anat", bufs=2) as apool,
            tc.tile_pool(name="small", bufs=4) as spool,
            tc.tile_pool(name="ostage", bufs=5) as opool,
            tc.tile_pool(name="ps", bufs=1, space="PSUM") as ps,
        ):
            # ---- persistent SBUF tensors ----
            wq_sb = pp.tile([128, DC, HD], bf16)
            wk_sb = pp.tile([128, DC, HD], bf16)
            wv_sb = pp.tile([128, DC, HD], bf16)
            wo_sb = pp.tile([128, 2, D], bf16)
            qT_sb = pp.tile([128, 2, T], bf16)   # [hd-in-pair, pair, t]
            kT_sb = pp.tile([128, 2, T], bf16)
            v1_sb = pp.tile([128, SC16, HPC, DK + 1], bf16)  # [s, sc, h, dk|1]
            aT_sb = pp.tile([128, 2, T], bf16)   # [hk-in-pair, pair, t]

            # first weights (DMA order = SP emission order)
            nc.sync.dma_start(out=wk_sb[:], in_=wk_r)
            nc.sync.dma_start(out=wq_sb[:], in_=wq_r)

            # ones column of v1 (sum-exp trick)
            nc.vector.memset(v1_sb[:, :, :, DK:DK + 1], 1.0)

            if has_bias:
                bq_sb = pp.tile([1, HD], bf16)
                bk_sb = pp.tile([1, HD], bf16)
                bv_sb = pp.tile([1, HD], bf16)
                ones_sb = pp.tile([1, 512], bf16)
                nc.sync.dma_start(out=bq_sb[0:1, :], in_=bqs.ap()[None, :])
                nc.sync.dma_start(out=bk_sb[0:1, :], in_=bks.ap()[None, :])
                nc.sync.dma_start(out=bv_sb[0:1, :], in_=bvs.ap()[None, :])
                nc.vector.memset(ones_sb[:], 1.0)
            b_sb = {"q": bq_sb, "k": bk_sb} if has_bias else {"q": None,
                                                             "k": None}

            # ---- x input tiles: [128, 1024] halves ----
            # xk: ring of 8, half1 recycles half0 (DMA emitted after the
            # half0 readers).  xq: 16 tiles, no recycling.  xv: ring of 8,
            # half1 recycles half0 (DMA emitted via aux unit after the v
            # units that read half0).
            xk_t = {0: [xpool.tile([128, 1024], bf16, tag="xk", bufs=8,
                                   name=f"xk0_{c}") for c in range(DC)]}
            xq_t = {0: [xpool.tile([128, 1024], bf16, tag="xq", bufs=8,
                                   name=f"xq0_{c}") for c in range(DC)]}
            for c in range(DC):
                nc.sync.dma_start(out=xk_t[0][c][:], in_=xk_r[c][:, 0:1024])
            for c in range(DC):
                nc.sync.dma_start(out=xq_t[0][c][:], in_=xq_r[c][:, 0:1024])

            # ---------------- emission units ----------------

            def proj_qk_unit(kind, p, tch, late=False):
                # one [128hd, 512t] psum tile of the q/k projection
                w_sb, dst_sb = ((wq_sb, qT_sb) if kind == "q"
                                else (wk_sb, kT_sb))
                def _emit():
                    pt = ps.tile([128, 512], f32, tag="mix", bufs=2,
                                 name="pqk")
                    xh = (xq_t if kind == "q" else xk_t)[tch // 2]
                    lsl = slice((tch % 2) * 512, (tch % 2) * 512 + 512)
                    tsl = slice(tch * 512, (tch + 1) * 512)
                    for c in range(DC):
                        nc.tensor.matmul(
                            pt[:],
                            w_sb[:, c, p * 128:(p + 1) * 128],
                            xh[c][:, lsl],
                            start=(c == 0),
                            stop=(c == DC - 1) and not has_bias,
                        )
                    if has_bias:
                        nc.tensor.matmul(
                            pt[:],
                            b_sb[kind][0:1, p * 128:(p + 1) * 128],
                            ones_sb[0:1, :],
                            start=False, stop=True,
                        )
                    nc.vector.tensor_copy(dst_sb[:, p, tsl], pt[:])

                def emit():
                    band = "c" if late else "p"
                    with tc.high_priority(offset=tc.cur_priority - prio[band]):
                        prio[band] += 32
                        _emit()
                return emit

            def proj_v_unit(sc):
                # one [128s, 256hd] psum tile of the v projection
                def _emit():
                    pt = ps.tile([128, HD], f32, tag="mix", bufs=2, name="pv")
                    xh = xv_t[sc // 8]
                    ssl = slice((sc % 8) * 128, (sc % 8) * 128 + 128)
                    for c in range(DC):
                        nc.tensor.matmul(
                            pt[:],
                            xh[c][:, ssl],
                            wv_sb[:, c, :],
                            start=(c == 0),
                            stop=(c == DC - 1) and not has_bias,
                        )
                    if has_bias:
                        nc.tensor.matmul(
                            pt[:], ones_sb[0:1, 0:128], bv_sb[0:1, :],
                            start=False, stop=True,
                        )
                    nc.vector.tensor_copy(
                        v1_sb[:, sc, :, 0:DK],
                        pt[:].rearrange("p (h k) -> p h k", h=HPC))

                def emit():
                    with tc.high_priority(offset=tc.cur_priority - prio["p"]):
                        prio["p"] += 32
                        _emit()
                return emit

            pts = {}     # probs tiles keyed (h, j, sc)
            att4 = {}    # psum accumulators keyed (h, j, q)
            anat = {}    # normalized attn tiles keyed (pair, j)

            prio = {"s": 0, "p": 500_000, "c": 1_000_000, "l": 1_500_000}

            def scores_unit(h, j, sc):
                # [128s, 1024t] scores psum tile + its exp; band-0 priority
                # so the scheduler always prefers feeding the Act engine
                p, o = h // 2, (h % 2) * 64
                with tc.high_priority(offset=tc.cur_priority - prio["s"]):
                    prio["s"] += 32
                    st = ps.tile([128, 1024], f32, tag="sc", bufs=2,
                                 name="st")
                    for th in range(2):
                        tsl = slice(j * 1024 + th * 512,
                                    j * 1024 + (th + 1) * 512)
                        nc.tensor.matmul(
                            st[:, th * 512:(th + 1) * 512],
                            kT_sb[o:o + 64, p, sc * 128:(sc + 1) * 128],
                            qT_sb[o:o + 64, p, tsl],
                            start=True, stop=True,
                        )
                    pt = ppool.tile([128, 1024], bf16, tag="pt", name="pt")
                    nc.scalar.activation(pt[:], st[:], AF.Exp)
                    pts[(h, j, sc)] = pt

            def attnv_chain(h, j, q, tt):
                # one full accumulation chain (16 matmuls) for t-subtile
                # q*4+tt.  Chains in the same psum tile must be sequential:
                # start_tensor_calc marks the whole 2KB zero-region pending,
                # so interleaved chains corrupt each other.
                def _emit():
                    if tt == 0:
                        att4[(h, j, q)] = ps.tile(
                            [128, 4, DK + 1], f32, tag="a4", bufs=2,
                            name="att4")
                    a4 = att4[(h, j, q)]
                    for sc in range(SC16):
                        nc.tensor.matmul(
                            a4[:, tt],
                            pts[(h, j, sc)][:, (q * 4 + tt) * 128:
                                            (q * 4 + tt + 1) * 128],
                            v1_sb[:, sc, h, :],
                            start=(sc == 0),
                            stop=(sc == SC16 - 1),
                        )
                    if (q, tt) == (1, 3):
                        for sc in range(SC16):
                            del pts[(h, j, sc)]

                def emit():
                    with tc.high_priority(offset=tc.cur_priority - prio["c"]):
                        prio["c"] += 32
                        _emit()
                return emit

            def norm_unit(h, j, q):
                # normalize one att4 half of (h, j) into anat[(pair, j)]
                pair, hi = h // 2, h % 2
                def _emit():
                    if (pair, j) not in anat:
                        anat[(pair, j)] = apool.tile(
                            [128, TT, 2, DK], bf16, tag="an", name="an")
                    an = anat[(pair, j)]
                    a4 = att4.pop((h, j, q))
                    rec = spool.tile([128, 4], f32, tag="rec", name="rec")
                    nc.vector.reciprocal(rec[:], a4[:, :, DK])
                    nc.vector.tensor_mul(
                        an[:, q * 4:(q + 1) * 4, hi, :],
                        a4[:, :, 0:DK],
                        rec[:, :, None].broadcast_to([128, 4, DK]),
                    )

                def emit():
                    with tc.high_priority(offset=tc.cur_priority - prio["c"]):
                        prio["c"] += 32
                        _emit()
                return emit

            def transpose_unit(pair, j):
                # 8 dma xbar transposes [128t,128hk] -> aT[hk, t]
                def emit():
                    an = anat.pop((pair, j))
                    for tt in range(TT):
                        nc.sync.dma_start(
                            out=aT_sb[:, pair,
                                      j * 1024 + tt * 128:
                                      j * 1024 + (tt + 1) * 128],
                            in_=an[:, tt, :, :],
                            transpose=True,
                        )
                return emit

            def outproj_unit(j, ti):
                # one t-tile of the output projection: [128t, 1024d]
                tg = j * TT + ti
                tag = "sc" if (j == 1 and ti % 2 == 1) else "mix"
                def emit():
                    po = ps.tile([128, 512], f32, tag=tag, bufs=2,
                                 name="po")
                    po2 = ps.tile([128, 512], f32, tag=tag, bufs=2,
                                  name="po2")
                    for dh, pot in ((0, po), (1, po2)):
                        for hp in range(2):
                            nc.tensor.matmul(
                                pot[:],
                                aT_sb[:, hp, tg * 128:(tg + 1) * 128],
                                wo_sb[:, hp, dh * 512:(dh + 1) * 512],
                                start=(hp == 0), stop=(hp == 1),
                            )
                    ob = opool.tile([128, D], bf16, tag="ob", name="ob")
                    nc.vector.tensor_copy(ob[:, 0:512], po[:])
                    if j == 1:
                        # Act engine is idle in the tail; split the drain
                        nc.scalar.copy(ob[:, 512:1024], po2[:])
                    else:
                        nc.vector.tensor_copy(ob[:, 512:1024], po2[:])
                    nc.sync.dma_start(
                        out=out.ap()[tg * 128:(tg + 1) * 128, :], in_=ob[:])
                return emit

            # ---------------- aux queue with pacing ----------------
            aux = deque()          # entries: (cost, name, emit_fn)
            emitted = set()
            budget = [0]

            def pump(n_rows):
                budget[0] = min(budget[0] + n_rows, AUX_CAP)
                while aux and aux[0][0] <= budget[0]:
                    cost, name, fn = aux.popleft()
                    budget[0] -= cost
                    emitted.add(name)
                    fn()

            def flush_until(name):
                if name in emitted:
                    return
                while aux:
                    cost, nm, fn = aux.popleft()
                    emitted.add(nm)
                    fn()
                    if nm == name:
                        return
                raise AssertionError(f"aux marker {name} not found")

            # ---------------- the stream ----------------
            # P1/P2: k proj t0/t1 (xk half0; pair1 last -- band-P priority
            # order lets pair0 + q p0 feed block 0 first), q p0 j0
            for tch in range(2):
                proj_qk_unit("k", 0, tch)()
            for tch in range(2):
                proj_qk_unit("q", 0, tch)()
            for tch in range(2):
                proj_qk_unit("k", 1, tch)()

            # xk half1 (recycles half0 slots -- emitted after readers above),
            # then xq j1, wv, xv (both halves, own slots), wo: everything
            # up-front in consumer order, no deferred DMAs.
            xk_t[1] = [xpool.tile([128, 1024], bf16, tag="xk", bufs=8,
                                  name=f"xk1_{c}") for c in range(DC)]
            for c in range(DC):
                nc.sync.dma_start(out=xk_t[1][c][:], in_=xk_r[c][:, 1024:2048])
            nc.sync.dma_start(out=wv_sb[:], in_=wv_r)
            xv_t = {h: [xpool.tile([128, 1024], bf16, tag="xv", bufs=16,
                                   name=f"xv{h}_{c}") for c in range(DC)]
                    for h in range(2)}
            for h in range(2):
                for c in range(DC):
                    nc.sync.dma_start(out=xv_t[h][c][:],
                                      in_=xv_r[c][:, h * 1024:(h + 1) * 1024])
            nc.sync.dma_start(out=wo_sb[:], in_=wo_r)

            # aux for block 0 and the early projections
            aux.append((4096, "qp1t0", proj_qk_unit("q", 1, 0)))
            aux.append((4096, "qp1t1", proj_qk_unit("q", 1, 1)))
            for sc in range(SC16):
                aux.append((2048, f"v{sc}", proj_v_unit(sc)))
            aux.append((4096, "qp0t2", proj_qk_unit("q", 0, 2, late=True)))
            aux.append((4096, "qp0t3", proj_qk_unit("q", 0, 3, late=True)))
            aux.append((4096, "qp1t2", proj_qk_unit("q", 1, 2, late=True)))
            aux.append((4096, "qp1t3", proj_qk_unit("q", 1, 3, late=True)))

            blocks = [(0, 0), (1, 0), (2, 0), (3, 0),
                      (0, 1), (1, 1), (2, 1), (3, 1)]
            chains = [(q, tt) for q in range(2) for tt in range(4)]

            def post_block(ph, pj):
                # prev block's chains are done: normalize the q1 half (q0
                # was normalized in-loop), then transpose / out-project when
                # a (pair, j) group completes
                aux.appendleft((0, f"norm_{ph}_{pj}_1", norm_unit(ph, pj, 1)))
                if ph % 2 == 1:
                    pair = ph // 2
                    aux.append((0, f"T_{pair}_{pj}",
                                transpose_unit(pair, pj)))
                    if pair == 1:
                        for ti in range(TT):
                            aux.append((2048, f"O_{pj}_{ti}",
                                        outproj_unit(pj, ti)))

            for bi, (h, j) in enumerate(blocks):
                # markers that must be emitted before this block's scores
                if bi == 2:
                    flush_until("qp1t1")   # also drains kp1t0..3
                if bi == 4:
                    flush_until("qp0t3")
                if bi == 6:
                    flush_until("qp1t3")
                for sc in range(SC16):
                    scores_unit(h, j, sc)
                    if bi == 0 and sc == 7:
                        # k proj t2/t3 (xk half1) before sc8 needs them
                        for tch in range(2, 4):
                            proj_qk_unit("k", 0, tch)()
                        for tch in range(2, 4):
                            proj_qk_unit("k", 1, tch)()
                        continue
                    lo = TT if bi == 1 else 4
                    if bi > 0 and lo <= sc < lo + TT:
                        # weave prev block's attnV chains into the scores
                        # stream (early in the block so the probs ring and
                        # att4 tiles free well before the next seam; block 1
                        # waits for sc8 because v1 lands late)
                        if bi == 1 and sc == TT:
                            flush_until("v15")  # chains read all of v1
                        ph, pj = blocks[bi - 1]
                        q, tt = chains[sc - lo]
                        if (q, tt) == (1, 0):
                            # free the q0 psum tile before q1's conclude
                            norm_unit(ph, pj, 0)()
                        attnv_chain(ph, pj, q, tt)()
                    pump(1500 if bi <= 1 else 1024)
                if bi == 0:
                    # xq j1 recycles the xk ring (readers: k proj, emitted
                    # above); not needed until block 4
                    xq_t[1] = [xpool.tile([128, 1024], bf16, tag="xk",
                                          bufs=8, name=f"xq1_{c}")
                               for c in range(DC)]
                    for c in range(DC):
                        nc.sync.dma_start(out=xq_t[1][c][:],
                                          in_=xq_r[c][:, 1024:2048])
                if bi > 0:
                    post_block(*blocks[bi - 1])

            # tail: last block's attnV uses 4 psum regions (the mix slots
            # are free by now) of 2 t-subtiles each, so only the second
            # chain of each region serializes after the final exp.
            rtiles = {}
            for i in range(2):
                for r in range(4):
                    g = r * 2 + i          # global t-subtile 0..7
                    with tc.high_priority(offset=tc.cur_priority
                                          - prio["c"]):
                        prio["c"] += 32
                        if i == 0:
                            rtiles[r] = ps.tile(
                                [128, 2, DK + 1], f32,
                                tag="a4" if r < 2 else "mix", bufs=2,
                                name=f"at{r}")
                        a4 = rtiles[r]
                        for sc in range(SC16):
                            nc.tensor.matmul(
                                a4[:, i],
                                pts[(3, 1, sc)][:, g * 128:(g + 1) * 128],
                                v1_sb[:, sc, 3, :],
                                start=(sc == 0), stop=(sc == SC16 - 1),
                            )
            for sc in range(SC16):
                del pts[(3, 1, sc)]
            an = anat[(1, 1)]          # created by norm(2, 1, *)
            for r in range(4):
                with tc.high_priority(offset=tc.cur_priority - prio["c"]):
                    prio["c"] += 32
                    a4 = rtiles[r]
                    rec = spool.tile([128, 2], f32, tag="rec", name="rec")
                    nc.vector.reciprocal(rec[:], a4[:, :, DK])
                    nc.vector.tensor_mul(
                        an[:, r * 2:(r + 1) * 2, 1, :],
                        a4[:, :, 0:DK],
                        rec[:, :, None].broadcast_to([128, 2, DK]),
                    )
            aux.append((0, "T_1_1", transpose_unit(1, 1)))
            for ti in range(TT):
                aux.append((2048, f"O_1_{ti}", outproj_unit(1, ti)))
            while aux:
                _, nm, fn = aux.popleft()
                emitted.add(nm)
                fn()

            if debug:
                nc.sync.dma_start(out=dbg["dkT"].ap(), in_=kT_sb[:])
                nc.sync.dma_start(out=dbg["dqT"].ap(), in_=qT_sb[:])
                nc.sync.dma_start(out=dbg["dv1"].ap(), in_=v1_sb[:])
                nc.sync.dma_start(out=dbg["daT"].ap(), in_=aT_sb[:])

    nc.compile()
    return nc


_NC_CACHE = {}


def get_nc(has_bias=False):
    key = ("nc", has_bias)
    if key not in _NC_CACHE:
        _NC_CACHE[key] = build_core(has_bias)
    return _NC_CACHE[key]


def make_in_maps(query, value, key, Wq, bq, Wk, bk, Wv, bv, Wo, bo,
                 has_bias):
    import ml_dtypes
    bf16 = ml_dtypes.bfloat16
    scale = np.float32(1.0 / np.sqrt(DK))
    xT = {}
    for b in range(B):
        xT[b] = {
            "q": np.ascontiguousarray(
                np.asarray(query[b], np.float32).T.astype(bf16)),
            "k": np.ascontiguousarray(
                np.asarray(key[b], np.float32).T.astype(bf16)),
            "v": np.ascontiguousarray(
                np.asarray(value[b], np.float32).T.astype(bf16)),
        }
    Wq_f = (np.asarray(Wq, np.float32) * scale).reshape(D, H * DK).astype(bf16)
    Wk_f = np.asarray(Wk, np.float32).reshape(D, H * DK).astype(bf16)
    Wv_f = np.asarray(Wv, np.float32).reshape(D, H * DK).astype(bf16)
    Wo_f = np.asarray(Wo, np.float32).reshape(H * DK, D).astype(bf16)
    bq_f = (np.asarray(bq, np.float32) * scale).reshape(H * DK).astype(bf16)
    bk_f = np.asarray(bk, np.float32).reshape(H * DK).astype(bf16)
    bv_f = np.asarray(bv, np.float32).reshape(H * DK).astype(bf16)
    in_maps = []
    for i in range(N_CORES):
        b = i // 4
        sl = slice((i % 4) * HD, (i % 4 + 1) * HD)
        m = {
            "xqT": xT[b]["q"],
            "xkT": xT[b]["k"],
            "xvT": xT[b]["v"],
            "wq": np.ascontiguousarray(Wq_f[:, sl]),
            "wk": np.ascontiguousarray(Wk_f[:, sl]),
            "wv": np.ascontiguousarray(Wv_f[:, sl]),
            "wo": np.ascontiguousarray(Wo_f[sl, :]),
        }
        if has_bias:
            m["bqs"] = np.ascontiguousarray(bq_f[sl])
            m["bks"] = np.ascontiguousarray(bk_f[sl])
            m["bvs"] = np.ascontiguousarray(bv_f[sl])
        in_maps.append(m)
    return in_maps


def gather(results, bo):
    out = np.zeros((B, T, D), np.float32)
    for i in range(N_CORES):
        out[i // 4] += np.asarray(results[i]["out"], np.float32)
    out += np.asarray(bo, np.float32)[None, None, :]
    return out


def kernel(query, value, key, Wq, bq, Wk, bk, Wv, bv, Wo, bo):
    from concourse.bass_utils import run_bass_kernel_spmd

    has_bias = bool(
        np.any(np.asarray(bq)) or np.any(np.asarray(bk))
        or np.any(np.asarray(bv)))
    nc = get_nc(has_bias)
    in_maps = make_in_maps(query, value, key, Wq, bq, Wk, bk, Wv, bv, Wo, bo,
                           has_bias)
    res = run_bass_kernel_spmd(nc, in_maps, list(range(N_CORES)))
    return gather(res.results, bo)


# revision 47
# speedup vs baseline: 1.0107x; 1.0020x over previous
"""Multi-head attention TRN2 kernel (v2, bf16 dataflow).

Problem: B=2, T=S=2048, D=1024, H=16, DK=64 (fp32 in/out).

Sharding (8 cores): core i handles batch b = i // 4 and the 4 heads
[4*(i%4), 4*(i%4)+4).  Each core computes q/k/v projections for its head
slice, attention over them, and a *partial* output projection (its heads'
rows of Wo).  The host sums the 4 partials per batch and adds bo.

v2 design (vs v1):
  - everything bf16 except psum accumulation (f32): halves DMA, enables
    1024-wide moving operands.
  - attnV in natural layout out[t, dk+1] (M=128 per matmul, N=65), with a
    ones-column in v giving sum(exp) for free in column 64.
  - probs = exp(scores) emitted bf16 straight to SBUF by the Act engine.
  - attn normalization on DVE (reciprocal of col 64, broadcast multiply).
  - attn[t, hk] -> attnT[hk, t] via DMA xbar transpose (dma_start_transpose).
  - PE instruction stream hand-woven: each scores tile is followed by the
    attnV matmuls of the tile 4 positions back plus token-bucket-paced
    "aux" units (projections, v-proj, out-proj) so the in-order PE queue
    never stalls behind the Act engine's exp drain.

Per-core layout (host pre-transposes / pre-slices / pre-scales / casts):
  xqT, xkT, xvT : (1024, 2048) bf16  -- x[b].T
  wq, wk, wv    : (1024, 256) bf16   -- W[:, h0:h0+4, :] (wq,bq pre-scaled)
  wo            : (256, 1024) bf16   -- Wo[h0:h0+4]
  out           : (2048, 1024) bf16  -- partial; host sums in f32, adds bo
"""

import numpy as np

B, T, S, D, H, DK = 2, 2048, 2048, 1024, 16, 64
HPC = 4            # heads per core
HD = HPC * DK      # 256 projected cols per core
N_CORES = 8
DC = D // 128      # 8 contraction chunks
SC16 = S // 128    # 16 s-chunks of 128
NJ = 2             # t-chunks of 1024 ("j blocks")
TT = 8             # t-subtiles of 128 per j block
LAG = 4            # attnV trails scores by this many s-chunks
PT_BUFS = 40       # probs ring (2 blocks + margin)
AUX_TILE = 1150    # aux matmul rows budget per scores tile
AUX_CAP = 3000


def build_core(has_bias=False, debug=False):
    import concourse.mybir as mybir
    from concourse import bacc
    from concourse.tile import TileContext
    from collections import deque

    dt = mybir.dt
    f32 = dt.float32
    bf16 = dt.bfloat16
    AF = mybir.ActivationFunctionType

    nc = bacc.Bacc("TRN2", target_bir_lowering=False, debug=False,
                   num_devices=N_CORES)

    xqT = nc.dram_tensor("xqT", [D, T], bf16, kind="ExternalInput")
    xkT = nc.dram_tensor("xkT", [D, T], bf16, kind="ExternalInput")
    xvT = nc.dram_tensor("xvT", [D, T], bf16, kind="ExternalInput")
    wq = nc.dram_tensor("wq", [D, HD], bf16, kind="ExternalInput")
    wk = nc.dram_tensor("wk", [D, HD], bf16, kind="ExternalInput")
    wv = nc.dram_tensor("wv", [D, HD], bf16, kind="ExternalInput")
    wo = nc.dram_tensor("wo", [HD, D], bf16, kind="ExternalInput")
    if has_bias:
        bqs = nc.dram_tensor("bqs", [HD], bf16, kind="ExternalInput")
        bks = nc.dram_tensor("bks", [HD], bf16, kind="ExternalInput")
        bvs = nc.dram_tensor("bvs", [HD], bf16, kind="ExternalInput")
    out = nc.dram_tensor("out", [T, D], bf16, kind="ExternalOutput")
    if debug:
        dbg = {
            nm: nc.dram_tensor(nm, shp, bf16, kind="ExternalOutput")
            for nm, shp in [
                ("dkT", [128, 2, T]), ("dqT", [128, 2, T]),
                ("dv1", [128, SC16, HPC, DK + 1]), ("daT", [128, 2, T]),
            ]}

    xq_r = xqT.ap().rearrange("(c p) t -> c p t", p=128)
    xk_r = xkT.ap().rearrange("(c p) t -> c p t", p=128)
    xv_r = xvT.ap().rearrange("(c p) t -> c p t", p=128)
    wq_r = wq.ap().rearrange("(c p) n -> p c n", p=128)
    wk_r = wk.ap().rearrange("(c p) n -> p c n", p=128)
    wv_r = wv.ap().rearrange("(c p) n -> p c n", p=128)
    wo_r = wo.ap().rearrange("(c p) n -> p c n", p=128)

    with TileContext(nc) as tc:
        tc.cur_priority = 2_000_000   # default band: aux/proj/outproj/DMA
        with (
            tc.tile_pool(name="persist", bufs=1) as pp,
            tc.tile_pool(name="xin", bufs=8) as xpool,
            tc.tile_pool(name="probs", bufs=PT_BUFS) as ppool,
            tc.tile_pool(name="anat", bufs=2) as apool,
            tc.tile_pool(name="small", bufs=4) as spool,
            tc.tile_pool(name="ostage", bufs=5) as opool,
            tc.tile_pool(name="ps", bufs=1, space="PSUM") as ps,
        ):
            # ---- persistent SBUF tensors ----
            wq_sb = pp.tile([128, DC, HD], bf16)
            wk_sb = pp.tile([128, DC, HD], bf16)
            wv_sb = pp.tile([128, DC, HD], bf16)
            wo_sb = pp.tile([128, 2, D], bf16)
            qT_sb = pp.tile([128, 2, T], bf16)   # [hd-in-pair, pair, t]
            kT_sb = pp.tile([128, 2, T], bf16)
            v1_sb = pp.tile([128, SC16, HPC, DK + 1], bf16)  # [s, sc, h, dk|1]
            aT_sb = pp.tile([128, 2, T], bf16)   # [hk-in-pair, pair, t]

            # first weights (DMA order = SP emission order)
            nc.sync.dma_start(out=wk_sb[:], in_=wk_r)
            nc.sync.dma_start(out=wq_sb[:], in_=wq_r)

            # ones column of v1 (sum-exp trick)
            nc.vector.memset(v1_sb[:, :, :, DK:DK + 1], 1.0)

            if has_bias:
                bq_sb = pp.tile([1, HD], bf16)
                bk_sb = pp.tile([1, HD], bf16)
                bv_sb = pp.tile([1, HD], bf16)
                ones_sb = pp.tile([1, 512], bf16)
                nc.sync.dma_start(out=bq_sb[0:1, :], in_=bqs.ap()[None, :])
                nc.sync.dma_start(out=bk_sb[0:1, :], in_=bks.ap()[None, :])
                nc.sync.dma_start(out=bv_sb[0:1, :], in_=bvs.ap()[None, :])
                nc.vector.memset(ones_sb[:], 1.0)
            b_sb = {"q": bq_sb, "k": bk_sb} if has_bias else {"q": None,
                                                             "k": None}

            # ---- x input tiles: [128, 1024] halves ----
            # xk: ring of 8, half1 recycles half0 (DMA emitted after the
            # half0 readers).  xq: 16 tiles, no recycling.  xv: ring of 8,
            # half1 recycles half0 (DMA emitted via aux unit after the v
            # units that read half0).
            xk_t = {0: [xpool.tile([128, 1024], bf16, tag="xk", bufs=8,
                                   name=f"xk0_{c}") for c in range(DC)]}
            xq_t = {0: [xpool.tile([128, 1024], bf16, tag="xq", bufs=8,
                                   name=f"xq0_{c}") for c in range(DC)]}
            for c in range(DC):
                nc.sync.dma_start(out=xk_t[0][c][:], in_=xk_r[c][:, 0:1024])
            for c in range(DC):
                nc.sync.dma_start(out=xq_t[0][c][:], in_=xq_r[c][:, 0:1024])

            # ---------------- emission units ----------------

            def proj_qk_unit(kind, p, tch, late=False):
                # one [128hd, 512t] psum tile of the q/k projection
                w_sb, dst_sb = ((wq_sb, qT_sb) if kind == "q"
                                else (wk_sb, kT_sb))
                def _emit():
                    pt = ps.tile([128, 512], f32, tag="mix", bufs=2,
                                 name="pqk")
                    xh = (xq_t if kind == "q" else xk_t)[tch // 2]
                    lsl = slice((tch % 2) * 512, (tch % 2) * 512 + 512)
                    tsl = slice(tch * 512, (tch + 1) * 512)
                    for c in range(DC):
                        nc.tensor.matmul(
                            pt[:],
                            w_sb[:, c, p * 128:(p + 1) * 128],
                            xh[c][:, lsl],
                            start=(c == 0),
                            stop=(c == DC - 1) and not has_bias,
                        )
                    if has_bias:
                        nc.tensor.matmul(
                            pt[:],
                            b_sb[kind][0:1, p * 128:(p + 1) * 128],
                            ones_sb[0:1, :],
                            start=False, stop=True,
                        )
                    nc.vector.tensor_copy(dst_sb[:, p, tsl], pt[:])

                def emit():
                    band = "c" if late else "p"
                    with tc.high_priority(offset=tc.cur_priority - prio[band]):
                        prio[band] += 32
                        _emit()
                return emit

            def proj_v_unit(sc):
                # one [128s, 256hd] psum tile of the v projection
                def _emit():
                    pt = ps.tile([128, HD], f32, tag="mix", bufs=2, name="pv")
                    xh = xv_t[sc // 8]
                    ssl = slice((sc % 8) * 128, (sc % 8) * 128 + 128)
                    for c in range(DC):
                        nc.tensor.matmul(
                            pt[:],
                            xh[c][:, ssl],
                            wv_sb[:, c, :],
                            start=(c == 0),
                            stop=(c == DC - 1) and not has_bias,
                        )
                    if has_bias:
                        nc.tensor.matmul(
                            pt[:], ones_sb[0:1, 0:128], bv_sb[0:1, :],
                            start=False, stop=True,
                        )
                    nc.vector.tensor_copy(
                        v1_sb[:, sc, :, 0:DK],
                        pt[:].rearrange("p (h k) -> p h k", h=HPC))

                def emit():
                    with tc.high_priority(offset=tc.cur_priority - prio["p"]):
                        prio["p"] += 32
                        _emit()
                return emit

            pts = {}     # probs tiles keyed (h, j, sc)
            att4 = {}    # psum accumulators keyed (h, j, q)
            anat = {}    # normalized attn tiles keyed (pair, j)

            prio = {"s": 0, "p": 500_000, "c": 1_000_000, "l": 1_500_000}

            def scores_unit(h, j, sc):
                # [128s, 1024t] scores psum tile + its exp; band-0 priority
                # so the scheduler always prefers feeding the Act engine
                p, o = h // 2, (h % 2) * 64
                with tc.high_priority(offset=tc.cur_priority - prio["s"]):
                    prio["s"] += 32
                    st = ps.tile([128, 1024], f32, tag="sc", bufs=2,
                                 name="st")
                    for th in range(2):
                        tsl = slice(j * 1024 + th * 512,
                                    j * 1024 + (th + 1) * 512)
                        nc.tensor.matmul(
                            st[:, th * 512:(th + 1) * 512],
                            kT_sb[o:o + 64, p, sc * 128:(sc + 1) * 128],
                            qT_sb[o:o + 64, p, tsl],
                            start=True, stop=True,
                        )
                    pt = ppool.tile([128, 1024], bf16, tag="pt", name="pt")
                    nc.scalar.activation(pt[:], st[:], AF.Exp)
                    pts[(h, j, sc)] = pt

            def attnv_chain(h, j, q, tt):
                # one full accumulation chain (16 matmuls) for t-subtile
                # q*4+tt.  Chains in the same psum tile must be sequential:
                # start_tensor_calc marks the whole 2KB zero-region pending,
                # so interleaved chains corrupt each other.
                def _emit():
                    if tt == 0:
                        att4[(h, j, q)] = ps.tile(
                            [128, 4, DK + 1], f32, tag="a4", bufs=2,
                            name="att4")
                    a4 = att4[(h, j, q)]
                    for sc in range(SC16):
                        nc.tensor.matmul(
                            a4[:, tt],
                            pts[(h, j, sc)][:, (q * 4 + tt) * 128:
                                            (q * 4 + tt + 1) * 128],
                            v1_sb[:, sc, h, :],
                            start=(sc == 0),
                            stop=(sc == SC16 - 1),
                        )
                    if (q, tt) == (1, 3):
                        for sc in range(SC16):
                            del pts[(h, j, sc)]

                def emit():
                    with tc.high_priority(offset=tc.cur_priority - prio["c"]):
                        prio["c"] += 32
                        _emit()
                return emit

            def norm_unit(h, j, q):
                # normalize one att4 half of (h, j) into anat[(pair, j)]
                pair, hi = h // 2, h % 2
                def _emit():
                    if (pair, j) not in anat:
                        anat[(pair, j)] = apool.tile(
                            [128, TT, 2, DK], bf16, tag="an", name="an")
                    an = anat[(pair, j)]
                    a4 = att4.pop((h, j, q))
                    rec = spool.tile([128, 4], f32, tag="rec", name="rec")
                    nc.vector.reciprocal(rec[:], a4[:, :, DK])
                    nc.vector.tensor_mul(
                        an[:, q * 4:(q + 1) * 4, hi, :],
                        a4[:, :, 0:DK],
                        rec[:, :, None].broadcast_to([128, 4, DK]),
                    )

                def emit():
                    with tc.high_priority(offset=tc.cur_priority - prio["c"]):
                        prio["c"] += 32
                        _emit()
                return emit

            def transpose_unit(pair, j):
                # 8 dma xbar transposes [128t,128hk] -> aT[hk, t]
                def emit():
                    an = anat.pop((pair, j))
                    for tt in range(TT):
                        nc.sync.dma_start(
                            out=aT_sb[:, pair,
                                      j * 1024 + tt * 128:
                                      j * 1024 + (tt + 1) * 128],
                            in_=an[:, tt, :, :],
                            transpose=True,
                        )
                return emit

            def outproj_unit(j, ti):
                # one t-tile of the output projection: [128t, 1024d]
                tg = j * TT + ti
                tag = "sc" if (j == 1 and ti % 2 == 1) else "mix"
                def emit():
                    po = ps.tile([128, 512], f32, tag=tag, bufs=2,
                                 name="po")
                    po2 = ps.tile([128, 512], f32, tag=tag, bufs=2,
                                  name="po2")
                    for dh, pot in ((0, po), (1, po2)):
                        for hp in range(2):
                            nc.tensor.matmul(
                                pot[:],
                                aT_sb[:, hp, tg * 128:(tg + 1) * 128],
                                wo_sb[:, hp, dh * 512:(dh + 1) * 512],
                                start=(hp == 0), stop=(hp == 1),
                            )
                    ob = opool.tile([128, D], bf16, tag="ob", name="ob")
                    nc.vector.tensor_copy(ob[:, 0:512], po[:])
                    if j == 1:
                        # Act engine is idle in the tail; split the drain
                        nc.scalar.copy(ob[:, 512:1024], po2[:])
                    else:
                        nc.vector.tensor_copy(ob[:, 512:1024], po2[:])
                    nc.sync.dma_start(
                        out=out.ap()[tg * 128:(tg + 1) * 128, :], in_=ob[:])
                return emit

            # ---------------- aux queue with pacing ----------------
            aux = deque()          # entries: (cost, name, emit_fn)
            emitted = set()
            budget = [0]

            def pump(n_rows):
                budget[0] = min(budget[0] + n_rows, AUX_CAP)
                while aux and aux[0][0] <= budget[0]:
                    cost, name, fn = aux.popleft()
                    budget[0] -= cost
                    emitted.add(name)
                    fn()

            def flush_until(name):
                if name in emitted:
                    return
                while aux:
                    cost, nm, fn = aux.popleft()
                    emitted.add(nm)
                    fn()
                    if nm == name:
                        return
                raise AssertionError(f"aux marker {name} not found")

            # ---------------- the stream ----------------
            # P1/P2: k proj t0/t1 (xk half0; pair1 last -- band-P priority
            # order lets pair0 + q p0 feed block 0 first), q p0 j0
            for tch in range(2):
                proj_qk_unit("k", 0, tch)()
            for tch in range(2):
                proj_qk_unit("q", 0, tch)()
            for tch in range(2):
                proj_qk_unit("k", 1, tch)()

            # xk half1 (recycles half0 slots -- emitted after readers above),
            # then xq j1, wv, xv (both halves, own slots), wo: everything
            # up-front in consumer order, no deferred DMAs.
            xk_t[1] = [xpool.tile([128, 1024], bf16, tag="xk", bufs=8,
                                  name=f"xk1_{c}") for c in range(DC)]
            for c in range(DC):
                nc.sync.dma_start(out=xk_t[1][c][:], in_=xk_r[c][:, 1024:2048])
            nc.sync.dma_start(out=wv_sb[:], in_=wv_r)
            xv_t = {h: [xpool.tile([128, 1024], bf16, tag="xv", bufs=16,
                                   name=f"xv{h}_{c}") for c in range(DC)]
                    for h in range(2)}
            for h in range(2):
                for c in range(DC):
                    nc.sync.dma_start(out=xv_t[h][c][:],
                                      in_=xv_r[c][:, h * 1024:(h + 1) * 1024])
            nc.sync.dma_start(out=wo_sb[:], in_=wo_r)

            # aux for block 0 and the early projections
            aux.append((4096, "qp1t0", proj_qk_unit("q", 1, 0)))
            aux.append((4096, "qp1t1", proj_qk_unit("q", 1, 1)))
            for sc in range(SC16):
                aux.append((2048, f"v{sc}", proj_v_unit(sc)))
            aux.append((4096, "qp0t2", proj_qk_unit("q", 0, 2, late=True)))
            aux.append((4096, "qp0t3", proj_qk_unit("q", 0, 3, late=True)))
            aux.append((4096, "qp1t2", proj_qk_unit("q", 1, 2, late=True)))
            aux.append((4096, "qp1t3", proj_qk_unit("q", 1, 3, late=True)))

            blocks = [(0, 0), (1, 0), (2, 0), (3, 0),
                      (0, 1), (1, 1), (2, 1), (3, 1)]
            chains = [(q, tt) for q in range(2) for tt in range(4)]

            def post_block(ph, pj):
                # prev block's chains are done: normalize the q1 half (q0
                # was normalized in-loop), then transpose / out-project when
                # a (pair, j) group completes
                aux.appendleft((0, f"norm_{ph}_{pj}_1", norm_unit(ph, pj, 1)))
                if ph % 2 == 1:
                    pair = ph // 2
                    aux.append((0, f"T_{pair}_{pj}",
                                transpose_unit(pair, pj)))
                    if pair == 1:
                        for ti in range(TT):
                            aux.append((2048, f"O_{pj}_{ti}",
                                        outproj_unit(pj, ti)))

            for bi, (h, j) in enumerate(blocks):
                # markers that must be emitted before this block's scores
                if bi == 2:
                    flush_until("qp1t1")   # also drains kp1t0..3
                if bi == 4:
                    flush_until("qp0t3")
                if bi == 6:
                    flush_until("qp1t3")
                rt7 = {}
                for sc in range(SC16):
                    scores_unit(h, j, sc)
                    if bi == 0 and sc == 7:
                        # k proj t2/t3 (xk half1) before sc8 needs them
                        for tch in range(2, 4):
                            proj_qk_unit("k", 0, tch)()
                        for tch in range(2, 4):
                            proj_qk_unit("k", 1, tch)()
                        continue
                    lo = TT if bi == 1 else 4
                    if bi == 7 and 4 <= sc < 12:
                        # prev block (2,1): 4-region attnV (the mix psum
                        # slots are free by now) -- only each region's
                        # second chain serializes behind the first four
                        idx = sc - 4
                        i, r = idx // 4, idx % 4
                        g = r * 2 + i
                        with tc.high_priority(offset=tc.cur_priority
                                              - prio["c"]):
                            prio["c"] += 32
                            if i == 0:
                                rt7[r] = ps.tile(
                                    [128, 2, DK + 1], f32,
                                    tag="a4" if r < 2 else "mix", bufs=2,
                                    name=f"b7r{r}")
                            a4 = rt7[r]
                            for psc in range(SC16):
                                nc.tensor.matmul(
                                    a4[:, i],
                                    pts[(2, 1, psc)][:, g * 128:
                                                     (g + 1) * 128],
                                    v1_sb[:, psc, 2, :],
                                    start=(psc == 0),
                                    stop=(psc == SC16 - 1),
                                )
                            if (i, r) == (1, 3):
                                for psc in range(SC16):
                                    del pts[(2, 1, psc)]
                    elif bi == 7 and 12 <= sc < 16:
                        r = sc - 12
                        with tc.high_priority(offset=tc.cur_priority
                                              - prio["c"]):
                            prio["c"] += 32
                            if (1, 1) not in anat:
                                anat[(1, 1)] = apool.tile(
                                    [128, TT, 2, DK], bf16, tag="an",
                                    name="an")
                            a4 = rt7[r]
                            rec = spool.tile([128, 2], f32, tag="rec",
                                             name="rec")
                            nc.vector.reciprocal(rec[:], a4[:, :, DK])
                            nc.vector.tensor_mul(
                                anat[(1, 1)][:, r * 2:(r + 1) * 2, 0, :],
                                a4[:, :, 0:DK],
                                rec[:, :, None].broadcast_to([128, 2, DK]),
                            )
                    elif bi > 0 and lo <= sc < lo + TT:
                        # weave prev block's attnV chains into the scores
                        # stream (early in the block so the probs ring and
                        # att4 tiles free well before the next seam; block 1
                        # waits for sc8 because v1 lands late)
                        if bi == 1 and sc == TT:
                            flush_until("v15")  # chains read all of v1
                        ph, pj = blocks[bi - 1]
                        q, tt = chains[sc - lo]
                        if (q, tt) == (1, 0):
                            # free the q0 psum tile before q1's conclude
                            norm_unit(ph, pj, 0)()
                        attnv_chain(ph, pj, q, tt)()
                    pump(1500 if bi <= 1 else 1024)
                if bi == 0:
                    # xq j1 recycles the xk ring (readers: k proj, emitted
                    # above); not needed until block 4
                    xq_t[1] = [xpool.tile([128, 1024], bf16, tag="xk",
                                          bufs=8, name=f"xq1_{c}")
                               for c in range(DC)]
                    for c in range(DC):
                        nc.sync.dma_start(out=xq_t[1][c][:],
                                          in_=xq_r[c][:, 1024:2048])
                if bi > 0 and bi != 7:
                    # (block 7's predecessor (2,1) was normalized in-loop)
                    post_block(*blocks[bi - 1])

            # tail: last block's attnV uses 4 psum regions (the mix slots
            # are free by now) of 2 t-subtiles each, so only the second
            # chain of each region serializes after the final exp.
            rtiles = {}
            for i in range(2):
                for r in range(4):
                    g = r * 2 + i          # global t-subtile 0..7
                    with tc.high_priority(offset=tc.cur_priority
                                          - prio["c"]):
                        prio["c"] += 32
                        if i == 0:
                            rtiles[r] = ps.tile(
                                [128, 2, DK + 1], f32,
                                tag="a4" if r < 2 else "mix", bufs=2,
                                name=f"at{r}")
                        a4 = rtiles[r]
                        for sc in range(SC16):
                            nc.tensor.matmul(
                                a4[:, i],
                                pts[(3, 1, sc)][:, g * 128:(g + 1) * 128],
                                v1_sb[:, sc, 3, :],
                                start=(sc == 0), stop=(sc == SC16 - 1),
                            )
            for sc in range(SC16):
                del pts[(3, 1, sc)]
            an = anat[(1, 1)]          # created by norm(2, 1, *)
            for r in range(4):
                with tc.high_priority(offset=tc.cur_priority - prio["c"]):
                    prio["c"] += 32
                    a4 = rtiles[r]
                    rec = spool.tile([128, 2], f32, tag="rec", name="rec")
                    nc.vector.reciprocal(rec[:], a4[:, :, DK])
                    nc.vector.tensor_mul(
                        an[:, r * 2:(r + 1) * 2, 1, :],
                        a4[:, :, 0:DK],
                        rec[:, :, None].broadcast_to([128, 2, DK]),
                    )
            aux.append((0, "T_1_1", transpose_unit(1, 1)))
            for ti in range(TT):
                aux.append((2048, f"O_1_{ti}", outproj_unit(1, ti)))
            while aux:
                _, nm, fn = aux.popleft()
                emitted.add(nm)
                fn()

            if debug:
                nc.sync.dma_start(out=dbg["dkT"].ap(), in_=kT_sb[:])
                nc.sync.dma_start(out=dbg["dqT"].ap(), in_=qT_sb[:])
                nc.sync.dma_start(out=dbg["dv1"].ap(), in_=v1_sb[:])
                nc.sync.dma_start(out=dbg["daT"].ap(), in_=aT_sb[:])

    nc.compile()
    return nc


_NC_CACHE = {}


def get_nc(has_bias=False):
    key = ("nc", has_bias)
    if key not in _NC_CACHE:
        _NC_CACHE[key] = build_core(has_bias)
    return _NC_CACHE[key]


def make_in_maps(query, value, key, Wq, bq, Wk, bk, Wv, bv, Wo, bo,
                 has_bias):
    import ml_dtypes
    bf16 = ml_dtypes.bfloat16
    scale = np.float32(1.0 / np.sqrt(DK))
    xT = {}
    for b in range(B):
        xT[b] = {
            "q": np.ascontiguousarray(
                np.asarray(query[b], np.float32).T.astype(bf16)),
            "k": np.ascontiguousarray(
                np.asarray(key[b], np.float32).T.astype(bf16)),
            "v": np.ascontiguousarray(
                np.asarray(value[b], np.float32).T.astype(bf16)),
        }
    Wq_f = (np.asarray(Wq, np.float32) * scale).reshape(D, H * DK).astype(bf16)
    Wk_f = np.asarray(Wk, np.float32).reshape(D, H * DK).astype(bf16)
    Wv_f = np.asarray(Wv, np.float32).reshape(D, H * DK).astype(bf16)
    Wo_f = np.asarray(Wo, np.float32).reshape(H * DK, D).astype(bf16)
    bq_f = (np.asarray(bq, np.float32) * scale).reshape(H * DK).astype(bf16)
    bk_f = np.asarray(bk, np.float32).reshape(H * DK).astype(bf16)
    bv_f = np.asarray(bv, np.float32).reshape(H * DK).astype(bf16)
    in_maps = []
    for i in range(N_CORES):
        b = i // 4
        sl = slice((i % 4) * HD, (i % 4 + 1) * HD)
        m = {
            "xqT": xT[b]["q"],
            "xkT": xT[b]["k"],
            "xvT": xT[b]["v"],
            "wq": np.ascontiguousarray(Wq_f[:, sl]),
            "wk": np.ascontiguousarray(Wk_f[:, sl]),
            "wv": np.ascontiguousarray(Wv_f[:, sl]),
            "wo": np.ascontiguousarray(Wo_f[sl, :]),
        }
        if has_bias:
            m["bqs"] = np.ascontiguousarray(bq_f[sl])
            m["bks"] = np.ascontiguousarray(bk_f[sl])
            m["bvs"] = np.ascontiguousarray(bv_f[sl])
        in_maps.append(m)
    return in_maps


def gather(results, bo):
    out = np.zeros((B, T, D), np.float32)
    for i in range(N_CORES):
        out[i // 4] += np.asarray(results[i]["out"], np.float32)
    out += np.asarray(bo, np.float32)[None, None, :]
    return out


def kernel(query, value, key, Wq, bq, Wk, bk, Wv, bv, Wo, bo):
    from concourse.bass_utils import run_bass_kernel_spmd

    has_bias = bool(
        np.any(np.asarray(bq)) or np.any(np.asarray(bk))
        or np.any(np.asarray(bv)))
    nc = get_nc(has_bias)
    in_maps = make_in_maps(query, value, key, Wq, bq, Wk, bk, Wv, bv, Wo, bo,
                           has_bias)
    res = run_bass_kernel_spmd(nc, in_maps, list(range(N_CORES)))
    return gather(res.results, bo)


# revision 48
# speedup vs baseline: 1.0109x; 1.0002x over previous
"""Multi-head attention TRN2 kernel (v2, bf16 dataflow).

Problem: B=2, T=S=2048, D=1024, H=16, DK=64 (fp32 in/out).

Sharding (8 cores): core i handles batch b = i // 4 and the 4 heads
[4*(i%4), 4*(i%4)+4).  Each core computes q/k/v projections for its head
slice, attention over them, and a *partial* output projection (its heads'
rows of Wo).  The host sums the 4 partials per batch and adds bo.

v2 design (vs v1):
  - everything bf16 except psum accumulation (f32): halves DMA, enables
    1024-wide moving operands.
  - attnV in natural layout out[t, dk+1] (M=128 per matmul, N=65), with a
    ones-column in v giving sum(exp) for free in column 64.
  - probs = exp(scores) emitted bf16 straight to SBUF by the Act engine.
  - attn normalization on DVE (reciprocal of col 64, broadcast multiply).
  - attn[t, hk] -> attnT[hk, t] via DMA xbar transpose (dma_start_transpose).
  - PE instruction stream hand-woven: each scores tile is followed by the
    attnV matmuls of the tile 4 positions back plus token-bucket-paced
    "aux" units (projections, v-proj, out-proj) so the in-order PE queue
    never stalls behind the Act engine's exp drain.

Per-core layout (host pre-transposes / pre-slices / pre-scales / casts):
  xqT, xkT, xvT : (1024, 2048) bf16  -- x[b].T
  wq, wk, wv    : (1024, 256) bf16   -- W[:, h0:h0+4, :] (wq,bq pre-scaled)
  wo            : (256, 1024) bf16   -- Wo[h0:h0+4]
  out           : (2048, 1024) bf16  -- partial; host sums in f32, adds bo
"""

import numpy as np

B, T, S, D, H, DK = 2, 2048, 2048, 1024, 16, 64
HPC = 4            # heads per core
HD = HPC * DK      # 256 projected cols per core
N_CORES = 8
DC = D // 128      # 8 contraction chunks
SC16 = S // 128    # 16 s-chunks of 128
NJ = 2             # t-chunks of 1024 ("j blocks")
TT = 8             # t-subtiles of 128 per j block
LAG = 4            # attnV trails scores by this many s-chunks
PT_BUFS = 40       # probs ring (2 blocks + margin)
AUX_TILE = 1150    # aux matmul rows budget per scores tile
AUX_CAP = 3000


def build_core(has_bias=False, debug=False):
    import concourse.mybir as mybir
    from concourse import bacc
    from concourse.tile import TileContext
    from collections import deque

    dt = mybir.dt
    f32 = dt.float32
    bf16 = dt.bfloat16
    AF = mybir.ActivationFunctionType

    nc = bacc.Bacc("TRN2", target_bir_lowering=False, debug=False,
                   num_devices=N_CORES)

    xqT = nc.dram_tensor("xqT", [D, T], bf16, kind="ExternalInput")
    xkT = nc.dram_tensor("xkT", [D, T], bf16, kind="ExternalInput")
    xvT = nc.dram_tensor("xvT", [D, T], bf16, kind="ExternalInput")
    wq = nc.dram_tensor("wq", [D, HD], bf16, kind="ExternalInput")
    wk = nc.dram_tensor("wk", [D, HD], bf16, kind="ExternalInput")
    wv = nc.dram_tensor("wv", [D, HD], bf16, kind="ExternalInput")
    wo = nc.dram_tensor("wo", [HD, D], bf16, kind="ExternalInput")
    if has_bias:
        bqs = nc.dram_tensor("bqs", [HD], bf16, kind="ExternalInput")
        bks = nc.dram_tensor("bks", [HD], bf16, kind="ExternalInput")
        bvs = nc.dram_tensor("bvs", [HD], bf16, kind="ExternalInput")
    out = nc.dram_tensor("out", [T, D], bf16, kind="ExternalOutput")
    if debug:
        dbg = {
            nm: nc.dram_tensor(nm, shp, bf16, kind="ExternalOutput")
            for nm, shp in [
                ("dkT", [128, 2, T]), ("dqT", [128, 2, T]),
                ("dv1", [128, SC16, HPC, DK + 1]), ("daT", [128, 2, T]),
            ]}

    xq_r = xqT.ap().rearrange("(c p) t -> c p t", p=128)
    xk_r = xkT.ap().rearrange("(c p) t -> c p t", p=128)
    xv_r = xvT.ap().rearrange("(c p) t -> c p t", p=128)
    wq_r = wq.ap().rearrange("(c p) n -> p c n", p=128)
    wk_r = wk.ap().rearrange("(c p) n -> p c n", p=128)
    wv_r = wv.ap().rearrange("(c p) n -> p c n", p=128)
    wo_r = wo.ap().rearrange("(c p) n -> p c n", p=128)

    with TileContext(nc) as tc:
        tc.cur_priority = 2_000_000   # default band: aux/proj/outproj/DMA
        with (
            tc.tile_pool(name="persist", bufs=1) as pp,
            tc.tile_pool(name="xin", bufs=8) as xpool,
            tc.tile_pool(name="probs", bufs=PT_BUFS) as ppool,
            tc.tile_pool(name="anat", bufs=2) as apool,
            tc.tile_pool(name="small", bufs=4) as spool,
            tc.tile_pool(name="ostage", bufs=5) as opool,
            tc.tile_pool(name="ps", bufs=1, space="PSUM") as ps,
        ):
            # ---- persistent SBUF tensors ----
            wq_sb = pp.tile([128, DC, HD], bf16)
            wk_sb = pp.tile([128, DC, HD], bf16)
            wv_sb = pp.tile([128, DC, HD], bf16)
            wo_sb = pp.tile([128, 2, D], bf16)
            qT_sb = pp.tile([128, 2, T], bf16)   # [hd-in-pair, pair, t]
            kT_sb = pp.tile([128, 2, T], bf16)
            v1_sb = pp.tile([128, SC16, HPC, DK + 1], bf16)  # [s, sc, h, dk|1]
            aT_sb = pp.tile([128, 2, T], bf16)   # [hk-in-pair, pair, t]

            # first weights (DMA order = SP emission order)
            nc.sync.dma_start(out=wk_sb[:], in_=wk_r)
            nc.sync.dma_start(out=wq_sb[:], in_=wq_r)

            # ones column of v1 (sum-exp trick)
            nc.vector.memset(v1_sb[:, :, :, DK:DK + 1], 1.0)

            if has_bias:
                bq_sb = pp.tile([1, HD], bf16)
                bk_sb = pp.tile([1, HD], bf16)
                bv_sb = pp.tile([1, HD], bf16)
                ones_sb = pp.tile([1, 512], bf16)
                nc.sync.dma_start(out=bq_sb[0:1, :], in_=bqs.ap()[None, :])
                nc.sync.dma_start(out=bk_sb[0:1, :], in_=bks.ap()[None, :])
                nc.sync.dma_start(out=bv_sb[0:1, :], in_=bvs.ap()[None, :])
                nc.vector.memset(ones_sb[:], 1.0)
            b_sb = {"q": bq_sb, "k": bk_sb} if has_bias else {"q": None,
                                                             "k": None}

            # ---- x input tiles: [128, 1024] halves ----
            # xk: ring of 8, half1 recycles half0 (DMA emitted after the
            # half0 readers).  xq: 16 tiles, no recycling.  xv: ring of 8,
            # half1 recycles half0 (DMA emitted via aux unit after the v
            # units that read half0).
            xk_t = {0: [xpool.tile([128, 1024], bf16, tag="xk", bufs=8,
                                   name=f"xk0_{c}") for c in range(DC)]}
            xq_t = {0: [xpool.tile([128, 1024], bf16, tag="xq", bufs=8,
                                   name=f"xq0_{c}") for c in range(DC)]}
            for c in range(DC):
                nc.sync.dma_start(out=xk_t[0][c][:], in_=xk_r[c][:, 0:1024])
            for c in range(DC):
                nc.sync.dma_start(out=xq_t[0][c][:], in_=xq_r[c][:, 0:1024])

            # ---------------- emission units ----------------

            def proj_qk_unit(kind, p, tch, late=False):
                # one [128hd, 512t] psum tile of the q/k projection
                w_sb, dst_sb = ((wq_sb, qT_sb) if kind == "q"
                                else (wk_sb, kT_sb))
                def _emit():
                    pt = ps.tile([128, 512], f32, tag="mix", bufs=2,
                                 name="pqk")
                    xh = (xq_t if kind == "q" else xk_t)[tch // 2]
                    lsl = slice((tch % 2) * 512, (tch % 2) * 512 + 512)
                    tsl = slice(tch * 512, (tch + 1) * 512)
                    for c in range(DC):
                        nc.tensor.matmul(
                            pt[:],
                            w_sb[:, c, p * 128:(p + 1) * 128],
                            xh[c][:, lsl],
                            start=(c == 0),
                            stop=(c == DC - 1) and not has_bias,
                        )
                    if has_bias:
                        nc.tensor.matmul(
                            pt[:],
                            b_sb[kind][0:1, p * 128:(p + 1) * 128],
                            ones_sb[0:1, :],
                            start=False, stop=True,
                        )
                    nc.vector.tensor_copy(dst_sb[:, p, tsl], pt[:])

                def emit():
                    band = "c" if late else "p"
                    with tc.high_priority(offset=tc.cur_priority - prio[band]):
                        prio[band] += 32
                        _emit()
                return emit

            def proj_v_unit(sc):
                # one [128s, 256hd] psum tile of the v projection
                def _emit():
                    pt = ps.tile([128, HD], f32, tag="mix", bufs=2, name="pv")
                    xh = xv_t[sc // 8]
                    ssl = slice((sc % 8) * 128, (sc % 8) * 128 + 128)
                    for c in range(DC):
                        nc.tensor.matmul(
                            pt[:],
                            xh[c][:, ssl],
                            wv_sb[:, c, :],
                            start=(c == 0),
                            stop=(c == DC - 1) and not has_bias,
                        )
                    if has_bias:
                        nc.tensor.matmul(
                            pt[:], ones_sb[0:1, 0:128], bv_sb[0:1, :],
                            start=False, stop=True,
                        )
                    nc.vector.tensor_copy(
                        v1_sb[:, sc, :, 0:DK],
                        pt[:].rearrange("p (h k) -> p h k", h=HPC))

                def emit():
                    with tc.high_priority(offset=tc.cur_priority - prio["p"]):
                        prio["p"] += 32
                        _emit()
                return emit

            pts = {}     # probs tiles keyed (h, j, sc)
            att4 = {}    # psum accumulators keyed (h, j, q)
            anat = {}    # normalized attn tiles keyed (pair, j)

            prio = {"s": 0, "p": 500_000, "c": 1_000_000, "l": 1_500_000}

            def scores_unit(h, j, sc):
                # [128s, 1024t] scores psum tile + its exp; band-0 priority
                # so the scheduler always prefers feeding the Act engine
                p, o = h // 2, (h % 2) * 64
                with tc.high_priority(offset=tc.cur_priority - prio["s"]):
                    prio["s"] += 32
                    st = ps.tile([128, 1024], f32, tag="sc", bufs=2,
                                 name="st")
                    for th in range(2):
                        tsl = slice(j * 1024 + th * 512,
                                    j * 1024 + (th + 1) * 512)
                        nc.tensor.matmul(
                            st[:, th * 512:(th + 1) * 512],
                            kT_sb[o:o + 64, p, sc * 128:(sc + 1) * 128],
                            qT_sb[o:o + 64, p, tsl],
                            start=True, stop=True,
                        )
                    pt = ppool.tile([128, 1024], bf16, tag="pt", name="pt")
                    nc.scalar.activation(pt[:], st[:], AF.Exp)
                    pts[(h, j, sc)] = pt

            def attnv_chain(h, j, q, tt):
                # one full accumulation chain (16 matmuls) for t-subtile
                # q*4+tt.  Chains in the same psum tile must be sequential:
                # start_tensor_calc marks the whole 2KB zero-region pending,
                # so interleaved chains corrupt each other.
                def _emit():
                    if tt == 0:
                        att4[(h, j, q)] = ps.tile(
                            [128, 4, DK + 1], f32, tag="a4", bufs=2,
                            name="att4")
                    a4 = att4[(h, j, q)]
                    for sc in range(SC16):
                        nc.tensor.matmul(
                            a4[:, tt],
                            pts[(h, j, sc)][:, (q * 4 + tt) * 128:
                                            (q * 4 + tt + 1) * 128],
                            v1_sb[:, sc, h, :],
                            start=(sc == 0),
                            stop=(sc == SC16 - 1),
                        )
                    if (q, tt) == (1, 3):
                        for sc in range(SC16):
                            del pts[(h, j, sc)]

                def emit():
                    with tc.high_priority(offset=tc.cur_priority - prio["c"]):
                        prio["c"] += 32
                        _emit()
                return emit

            def norm_unit(h, j, q):
                # normalize one att4 half of (h, j) into anat[(pair, j)]
                pair, hi = h // 2, h % 2
                def _emit():
                    if (pair, j) not in anat:
                        anat[(pair, j)] = apool.tile(
                            [128, TT, 2, DK], bf16, tag="an", name="an")
                    an = anat[(pair, j)]
                    a4 = att4.pop((h, j, q))
                    rec = spool.tile([128, 4], f32, tag="rec", name="rec")
                    nc.vector.reciprocal(rec[:], a4[:, :, DK])
                    nc.vector.tensor_mul(
                        an[:, q * 4:(q + 1) * 4, hi, :],
                        a4[:, :, 0:DK],
                        rec[:, :, None].broadcast_to([128, 4, DK]),
                    )

                def emit():
                    with tc.high_priority(offset=tc.cur_priority - prio["c"]):
                        prio["c"] += 32
                        _emit()
                return emit

            def transpose_unit(pair, j):
                # 8 dma xbar transposes [128t,128hk] -> aT[hk, t]
                def emit():
                    an = anat.pop((pair, j))
                    for tt in range(TT):
                        nc.sync.dma_start(
                            out=aT_sb[:, pair,
                                      j * 1024 + tt * 128:
                                      j * 1024 + (tt + 1) * 128],
                            in_=an[:, tt, :, :],
                            transpose=True,
                        )
                return emit

            def outproj_unit(j, ti):
                # one t-tile of the output projection: [128t, 1024d]
                tg = j * TT + ti
                tag = "sc" if (j == 1 and ti % 2 == 1) else "mix"
                def emit():
                    po = ps.tile([128, 512], f32, tag=tag, bufs=2,
                                 name="po")
                    po2 = ps.tile([128, 512], f32, tag=tag, bufs=2,
                                  name="po2")
                    for dh, pot in ((0, po), (1, po2)):
                        for hp in range(2):
                            nc.tensor.matmul(
                                pot[:],
                                aT_sb[:, hp, tg * 128:(tg + 1) * 128],
                                wo_sb[:, hp, dh * 512:(dh + 1) * 512],
                                start=(hp == 0), stop=(hp == 1),
                            )
                    ob = opool.tile([128, D], bf16, tag="ob", name="ob")
                    nc.vector.tensor_copy(ob[:, 0:512], po[:])
                    if j == 1:
                        # Act engine is idle in the tail; split the drain
                        nc.scalar.copy(ob[:, 512:1024], po2[:])
                    else:
                        nc.vector.tensor_copy(ob[:, 512:1024], po2[:])
                    nc.sync.dma_start(
                        out=out.ap()[tg * 128:(tg + 1) * 128, :], in_=ob[:])
                return emit

            # ---------------- aux queue with pacing ----------------
            aux = deque()          # entries: (cost, name, emit_fn)
            emitted = set()
            budget = [0]

            def pump(n_rows):
                budget[0] = min(budget[0] + n_rows, AUX_CAP)
                while aux and aux[0][0] <= budget[0]:
                    cost, name, fn = aux.popleft()
                    budget[0] -= cost
                    emitted.add(name)
                    fn()

            def flush_until(name):
                if name in emitted:
                    return
                while aux:
                    cost, nm, fn = aux.popleft()
                    emitted.add(nm)
                    fn()
                    if nm == name:
                        return
                raise AssertionError(f"aux marker {name} not found")

            # ---------------- the stream ----------------
            # P1/P2: k proj t0/t1 (xk half0; pair1 last -- band-P priority
            # order lets pair0 + q p0 feed block 0 first), q p0 j0
            for tch in range(2):
                proj_qk_unit("k", 0, tch)()
            for tch in range(2):
                proj_qk_unit("q", 0, tch)()
            for tch in range(2):
                proj_qk_unit("k", 1, tch)()

            # xk half1 (recycles half0 slots -- emitted after readers above),
            # then xq j1, wv, xv (both halves, own slots), wo: everything
            # up-front in consumer order, no deferred DMAs.
            xk_t[1] = [xpool.tile([128, 1024], bf16, tag="xk", bufs=8,
                                  name=f"xk1_{c}") for c in range(DC)]
            for c in range(DC):
                nc.sync.dma_start(out=xk_t[1][c][:], in_=xk_r[c][:, 1024:2048])
            nc.sync.dma_start(out=wv_sb[:], in_=wv_r)
            xv_t = {h: [xpool.tile([128, 1024], bf16, tag="xv", bufs=16,
                                   name=f"xv{h}_{c}") for c in range(DC)]
                    for h in range(2)}
            for h in range(2):
                for c in range(DC):
                    nc.sync.dma_start(out=xv_t[h][c][:],
                                      in_=xv_r[c][:, h * 1024:(h + 1) * 1024])
            nc.sync.dma_start(out=wo_sb[:], in_=wo_r)

            # aux for block 0 and the early projections
            aux.append((4096, "qp1t0", proj_qk_unit("q", 1, 0)))
            aux.append((4096, "qp1t1", proj_qk_unit("q", 1, 1)))
            for sc in range(SC16):
                aux.append((2048, f"v{sc}", proj_v_unit(sc)))
            aux.append((4096, "qp0t2", proj_qk_unit("q", 0, 2, late=True)))
            aux.append((4096, "qp0t3", proj_qk_unit("q", 0, 3, late=True)))
            aux.append((4096, "qp1t2", proj_qk_unit("q", 1, 2, late=True)))
            aux.append((4096, "qp1t3", proj_qk_unit("q", 1, 3, late=True)))

            blocks = [(0, 0), (1, 0), (2, 0), (3, 0),
                      (0, 1), (1, 1), (2, 1), (3, 1)]
            chains = [(q, tt) for q in range(2) for tt in range(4)]

            def post_block(ph, pj):
                # prev block's chains are done: normalize the q1 half (q0
                # was normalized in-loop), then transpose / out-project when
                # a (pair, j) group completes
                aux.appendleft((0, f"norm_{ph}_{pj}_1", norm_unit(ph, pj, 1)))
                if ph % 2 == 1:
                    pair = ph // 2
                    aux.append((0, f"T_{pair}_{pj}",
                                transpose_unit(pair, pj)))
                    if pair == 1:
                        for ti in range(TT):
                            aux.append((2048, f"O_{pj}_{ti}",
                                        outproj_unit(pj, ti)))

            for bi, (h, j) in enumerate(blocks):
                # markers that must be emitted before this block's scores
                if bi == 2:
                    flush_until("qp1t1")   # also drains kp1t0..3
                if bi == 4:
                    flush_until("qp0t3")
                if bi == 6:
                    flush_until("qp1t3")
                rt7 = {}
                for sc in range(SC16):
                    scores_unit(h, j, sc)
                    if bi == 0 and sc == 7:
                        # k proj t2/t3 (xk half1) before sc8 needs them
                        for tch in range(2, 4):
                            proj_qk_unit("k", 0, tch)()
                        for tch in range(2, 4):
                            proj_qk_unit("k", 1, tch)()
                        continue
                    lo = TT if bi == 1 else 4
                    if bi in (6, 7) and 4 <= sc < 12:
                        # prev block: 4-region attnV (the mix psum slots
                        # are free in blocks 6/7) -- only each region's
                        # second chain serializes behind the first four
                        ph, pj = blocks[bi - 1]
                        idx = sc - 4
                        i, r = idx // 4, idx % 4
                        g = r * 2 + i
                        with tc.high_priority(offset=tc.cur_priority
                                              - prio["c"]):
                            prio["c"] += 32
                            if i == 0:
                                rt7[r] = ps.tile(
                                    [128, 2, DK + 1], f32,
                                    tag="a4" if r < 2 else "mix", bufs=2,
                                    name=f"b{bi}r{r}")
                            a4 = rt7[r]
                            for psc in range(SC16):
                                nc.tensor.matmul(
                                    a4[:, i],
                                    pts[(ph, pj, psc)][:, g * 128:
                                                       (g + 1) * 128],
                                    v1_sb[:, psc, ph, :],
                                    start=(psc == 0),
                                    stop=(psc == SC16 - 1),
                                )
                            if (i, r) == (1, 3):
                                for psc in range(SC16):
                                    del pts[(ph, pj, psc)]
                    elif bi in (6, 7) and 12 <= sc < 16:
                        ph, pj = blocks[bi - 1]
                        pair, hi = ph // 2, ph % 2
                        r = sc - 12
                        with tc.high_priority(offset=tc.cur_priority
                                              - prio["c"]):
                            prio["c"] += 32
                            if (pair, pj) not in anat:
                                anat[(pair, pj)] = apool.tile(
                                    [128, TT, 2, DK], bf16, tag="an",
                                    name="an")
                            a4 = rt7[r]
                            rec = spool.tile([128, 2], f32, tag="rec",
                                             name="rec")
                            nc.vector.reciprocal(rec[:], a4[:, :, DK])
                            nc.vector.tensor_mul(
                                anat[(pair, pj)][:, r * 2:(r + 1) * 2,
                                                 hi, :],
                                a4[:, :, 0:DK],
                                rec[:, :, None].broadcast_to([128, 2, DK]),
                            )
                    elif bi > 0 and lo <= sc < lo + TT:
                        # weave prev block's attnV chains into the scores
                        # stream (early in the block so the probs ring and
                        # att4 tiles free well before the next seam; block 1
                        # waits for sc8 because v1 lands late)
                        if bi == 1 and sc == TT:
                            flush_until("v15")  # chains read all of v1
                        ph, pj = blocks[bi - 1]
                        q, tt = chains[sc - lo]
                        if (q, tt) == (1, 0):
                            # free the q0 psum tile before q1's conclude
                            norm_unit(ph, pj, 0)()
                        attnv_chain(ph, pj, q, tt)()
                    pump(1500 if bi <= 1 else 1024)
                if bi == 0:
                    # xq j1 recycles the xk ring (readers: k proj, emitted
                    # above); not needed until block 4
                    xq_t[1] = [xpool.tile([128, 1024], bf16, tag="xk",
                                          bufs=8, name=f"xq1_{c}")
                               for c in range(DC)]
                    for c in range(DC):
                        nc.sync.dma_start(out=xq_t[1][c][:],
                                          in_=xq_r[c][:, 1024:2048])
                if bi > 0 and bi not in (6, 7):
                    post_block(*blocks[bi - 1])
                elif bi == 6:
                    # (1,1) was normalized in-loop; still emit its transpose
                    aux.append((0, "T_0_1", transpose_unit(0, 1)))

            # tail: last block's attnV uses 4 psum regions (the mix slots
            # are free by now) of 2 t-subtiles each, so only the second
            # chain of each region serializes after the final exp.
            rtiles = {}
            for i in range(2):
                for r in range(4):
                    g = r * 2 + i          # global t-subtile 0..7
                    with tc.high_priority(offset=tc.cur_priority
                                          - prio["c"]):
                        prio["c"] += 32
                        if i == 0:
                            rtiles[r] = ps.tile(
                                [128, 2, DK + 1], f32,
                                tag="a4" if r < 2 else "mix", bufs=2,
                                name=f"at{r}")
                        a4 = rtiles[r]
                        for sc in range(SC16):
                            nc.tensor.matmul(
                                a4[:, i],
                                pts[(3, 1, sc)][:, g * 128:(g + 1) * 128],
                                v1_sb[:, sc, 3, :],
                                start=(sc == 0), stop=(sc == SC16 - 1),
                            )
            for sc in range(SC16):
                del pts[(3, 1, sc)]
            an = anat[(1, 1)]          # created by norm(2, 1, *)
            for r in range(4):
                with tc.high_priority(offset=tc.cur_priority - prio["c"]):
                    prio["c"] += 32
                    a4 = rtiles[r]
                    rec = spool.tile([128, 2], f32, tag="rec", name="rec")
                    nc.vector.reciprocal(rec[:], a4[:, :, DK])
                    nc.vector.tensor_mul(
                        an[:, r * 2:(r + 1) * 2, 1, :],
                        a4[:, :, 0:DK],
                        rec[:, :, None].broadcast_to([128, 2, DK]),
                    )
            aux.append((0, "T_1_1", transpose_unit(1, 1)))
            for ti in range(TT):
                aux.append((2048, f"O_1_{ti}", outproj_unit(1, ti)))
            while aux:
                _, nm, fn = aux.popleft()
                emitted.add(nm)
                fn()

            if debug:
                nc.sync.dma_start(out=dbg["dkT"].ap(), in_=kT_sb[:])
                nc.sync.dma_start(out=dbg["dqT"].ap(), in_=qT_sb[:])
                nc.sync.dma_start(out=dbg["dv1"].ap(), in_=v1_sb[:])
                nc.sync.dma_start(out=dbg["daT"].ap(), in_=aT_sb[:])

    nc.compile()
    return nc


_NC_CACHE = {}


def get_nc(has_bias=False):
    key = ("nc", has_bias)
    if key not in _NC_CACHE:
        _NC_CACHE[key] = build_core(has_bias)
    return _NC_CACHE[key]


def make_in_maps(query, value, key, Wq, bq, Wk, bk, Wv, bv, Wo, bo,
                 has_bias):
    import ml_dtypes
    bf16 = ml_dtypes.bfloat16
    scale = np.float32(1.0 / np.sqrt(DK))
    xT = {}
    for b in range(B):
        xT[b] = {
            "q": np.ascontiguousarray(
                np.asarray(query[b], np.float32).T.astype(bf16)),
            "k": np.ascontiguousarray(
                np.asarray(key[b], np.float32).T.astype(bf16)),
            "v": np.ascontiguousarray(
                np.asarray(value[b], np.float32).T.astype(bf16)),
        }
    Wq_f = (np.asarray(Wq, np.float32) * scale).reshape(D, H * DK).astype(bf16)
    Wk_f = np.asarray(Wk, np.float32).reshape(D, H * DK).astype(bf16)
    Wv_f = np.asarray(Wv, np.float32).reshape(D, H * DK).astype(bf16)
    Wo_f = np.asarray(Wo, np.float32).reshape(H * DK, D).astype(bf16)
    bq_f = (np.asarray(bq, np.float32) * scale).reshape(H * DK).astype(bf16)
    bk_f = np.asarray(bk, np.float32).reshape(H * DK).astype(bf16)
    bv_f = np.asarray(bv, np.float32).reshape(H * DK).astype(bf16)
    in_maps = []
    for i in range(N_CORES):
        b = i // 4
        sl = slice((i % 4) * HD, (i % 4 + 1) * HD)
        m = {
            "xqT": xT[b]["q"],
            "xkT": xT[b]["k"],
            "xvT": xT[b]["v"],
            "wq": np.ascontiguousarray(Wq_f[:, sl]),
            "wk": np.ascontiguousarray(Wk_f[:, sl]),
            "wv": np.ascontiguousarray(Wv_f[:, sl]),
            "wo": np.ascontiguousarray(Wo_f[sl, :]),
        }
        if has_bias:
            m["bqs"] = np.ascontiguousarray(bq_f[sl])
            m["bks"] = np.ascontiguousarray(bk_f[sl])
            m["bvs"] = np.ascontiguousarray(bv_f[sl])
        in_maps.append(m)
    return in_maps


def gather(results, bo):
    out = np.zeros((B, T, D), np.float32)
    for i in range(N_CORES):
        out[i // 4] += np.asarray(results[i]["out"], np.float32)
    out += np.asarray(bo, np.float32)[None, None, :]
    return out


def kernel(query, value, key, Wq, bq, Wk, bk, Wv, bv, Wo, bo):
    from concourse.bass_utils import run_bass_kernel_spmd

    has_bias = bool(
        np.any(np.asarray(bq)) or np.any(np.asarray(bk))
        or np.any(np.asarray(bv)))
    nc = get_nc(has_bias)
    in_maps = make_in_maps(query, value, key, Wq, bq, Wk, bk, Wv, bv, Wo, bo,
                           has_bias)
    res = run_bass_kernel_spmd(nc, in_maps, list(range(N_CORES)))
    return gather(res.results, bo)
